# revision 29
# baseline (speedup 1.0000x reference)
"""Trainium2 Bass kernel for nn_CMambaSlim.

Strategy (8 NeuronCores):
  - Data-parallel trunk: each core runs the CMamba trunk (patch embed, 4
    mamba blocks, final RMSNorm) on B/8 = 4 batch samples, fp32/f32r.
  - AllGather of the flattened features (1 MB bf16) across the 8 cores.
  - Tensor-parallel output layer: core r streams rows [r*768, (r+1)*768) of
    out_W in bf16 (pre-transposed on host to [16000, 768]) and computes
    y[:, r*768:(r+1)*768]. out_b is added on the host during unsharding.

Schedule notes (CoreSim cost model):
  - All DMAs serialize on the DMA-engine device and hold the issuing
    engine's sequencer. SP's queue carries ONLY the wpack loads + the 32
    streamed weight chunks, so weight prefetch runs through the trunk and
    the AllGather. ccin/flatT/y DMAs issue from the Activation engine.
  - The residual stream h lives in PSUM: the out-projection matmuls
    accumulate straight into it (h' = h * 2^-l with the 2^-(l+1) folded
    into op_W host-side; rmsnorm is scale-invariant so only the eps
    constant needs a per-layer 4^-l).
  - The trunk is split into two independent 256-token halves (the conv
    windows are separated by a zeroed 4-column gap in hn), pipelined
    through ACT (square/sigmoid/aM), DVE (rstd/hn/u/gg/dab), Pool
    (scan/g0, SBUF-only operands), and PE.
  - ACT stays on the sigmoid table the whole trunk; rstd is computed on
    DVE as (ssum + D*eps_l)^-0.5 via AluOp.pow.
"""

import math
import os
import sys

import numpy as np

for _p in ("/opt/trn_rl_repo", "/root/.axon_site/_ro/trn_rl_repo"):
    if os.path.isdir(_p) and _p not in sys.path:
        sys.path.insert(0, _p)
        break

import concourse.bass as bass
import concourse.tile as tile
from concourse import mybir
from concourse.bass_utils import run_bass_kernel_spmd

# Model dims (hardcoded per problem spec)
B, C, L = 32, 64, 512
P, S = 16, 4
NP = 125
D = 128
INNER = 256
K5 = 5
NL = 4
F = 96
EPS = 1e-5

NCORES = 8
BLOC = B // NCORES            # 4 samples per core
OSL = (C * F) // NCORES       # 768 output cols per core
TOK = BLOC * 128              # padded token span (125 valid + 3 pad per sample)
HTOK = TOK // 2               # 256 tokens per pipelined half
HNW = 2 + HTOK + 4 + HTOK + 2  # hn with guards + inter-half gap = 520
LPAD = 520                    # x padded along L so the +8-shifted copy stays in bounds
NF = NP * D                   # 16000 contraction size
WKC = 4                       # k-chunks per weight-stream DMA
NQ = (NP + WKC - 1) // WKC    # 32 chunks (31 full + 1 partial)
WRING = 18                    # weight ring depth (chunks prefetchable)

f32 = mybir.dt.float32
f32r = mybir.dt.float32r
bf16 = mybir.dt.bfloat16
AF = mybir.ActivationFunctionType
OP = mybir.AluOpType

_PROG = None

SQRTD = math.sqrt(float(D))

# wph: bf16 embed inputs (x windows + patch-embed W), loaded first
NW = 129                                   # x windows (last is zero padding)
HOFF_PE8 = BLOC * NW * 4                   # 2064 cols of x
HCOLS = HOFF_PE8 + 8 * 128                 # + 1024 cols of patch-embed W
# wpack: fp32 consts (embed tail + one DMA per layer)
WOFF_IDN = TOK                             # 512 cols: posb broadcast to (b,k)
WOFF_SQD = WOFF_IDN + 128                  # 128 cols: identity matrix
WOFF_ONER = WOFF_SQD + 128                 # 128 cols: sqrt(D) everywhere
WOFF_MISC = WOFF_ONER + HTOK               # 256 cols: 1.0 (ones row)
WOFF_L0 = WOFF_MISC + 2                    # col 0: 1.0 (onesD), col 1: normf_w
LCOLS = K5 * INNER + INNER + 2 * D + 8 + INNER  # + 256 cols conv_b row
WCOLS = WOFF_L0 + NL * LCOLS


def build_program():
    nc = bass.Bass(num_devices=NCORES)

    wph = nc.declare_dram_parameter("wph", [128, HCOLS], bf16, isOutput=False)
    wpack = nc.declare_dram_parameter("wpack", [128, WCOLS], f32, isOutput=False)
    wt = nc.declare_dram_parameter("wt", [NF, OSL], bf16, isOutput=False)
    y = nc.declare_dram_parameter("y", [B, OSL], f32, isOutput=True)

    wtT = wt[:].tensor

    with tile.TileContext(nc) as tc:
        with (
            tc.tile_pool(name="const", bufs=1) as const,
            tc.tile_pool(name="work", bufs=1) as work,
            tc.tile_pool(name="wring", bufs=WRING) as wring,
            tc.tile_pool(name="ps", bufs=1, space="PSUM") as ps,
            tc.tile_pool(name="dram", bufs=1, space="DRAM") as dram,
        ):
            # ---------------- constant loads (embed parts, then per layer) ---
            # posbB/ident first (gates the first embed matmul), then x/pe8
            wp = const.tile([128, WCOLS], f32r)
            nc.sync.dma_start(out=wp[:, 0:WOFF_L0],
                              in_=wpack[:, 0:WOFF_L0].bitcast(f32r))
            wh = const.tile([128, HCOLS], bf16)
            nc.sync.dma_start(out=wh[:], in_=wph[:])
            for l in range(NL):
                c0 = WOFF_L0 + l * LCOLS
                nc.sync.dma_start(out=wp[:, c0:c0 + LCOLS],
                                  in_=wpack[:, c0:c0 + LCOLS].bitcast(f32r))

            xO4 = wh[:, 0:HOFF_PE8].rearrange(
                "p (b k s) -> p b k s", b=BLOC, s=4)          # [128, 4, 129, 4]
            pe8sb = wh[:, HOFF_PE8:HCOLS].rearrange("p (j d) -> p j d", j=8)
            posbB = wp[:, 0:WOFF_IDN]                          # [128, 512] (b,k)
            ident = wp[:, WOFF_IDN:WOFF_SQD]                   # I_128
            sqrtDrow = wp[0:1, WOFF_SQD:WOFF_SQD + 128]        # value sqrt(D)
            onesrow = wp[0:1, WOFF_ONER:WOFF_ONER + HTOK]      # value 1.0
            onesD = wp[:, WOFF_MISC:WOFF_MISC + 1]             # value 1.0
            normf = wp[:, WOFF_MISC + 1:WOFF_MISC + 2].bitcast(f32)

            def lview(l):
                b0 = WOFF_L0 + l * LCOLS
                w5 = wp[:, b0:b0 + K5 * INNER].rearrange(
                    "p (k i) -> p k i", k=K5)
                bw = wp[:, b0 + K5 * INNER:b0 + K5 * INNER + INNER]
                ow = wp[:, b0 + K5 * INNER + INNER:
                        b0 + K5 * INNER + INNER + 2 * D].rearrange(
                    "p (c d) -> p c d", c=2)
                scal = wp[:, b0 + LCOLS - 8 - INNER:
                          b0 + LCOLS - INNER].bitcast(f32).rearrange(
                    "p (s c) -> p s c", s=4)
                cbrow = wp[0:1, b0 + LCOLS - INNER:b0 + LCOLS]  # conv_b row
                return w5, bw, ow, scal, cbrow

            # mask01: 1 everywhere, 0 at each sample's k=0 column (scan reset)
            mask01 = const.tile([128, TOK], f32)
            nc.vector.memset(mask01[:], 1.0)
            for bq in range(BLOC):
                nc.vector.memset(mask01[:, bq * 128:bq * 128 + 1], 0.0)

            # residual stream h' lives in PSUM; out-projections accumulate
            # into it (never stopped). h' = h * 2^-l, exact via scaled op_W.
            hps = ps.tile([128, TOK], f32, tag="h", name="t_h")
            hps_bk = hps[:].rearrange("p (b k) -> p b k", b=BLOC)

            # normalized-input tile: [2 guard | half0 | 4 gap | half1 | 2 guard]
            hn = const.tile([128, HNW], f32)
            nc.vector.memset(hn[:, 0:2], 0.0)
            nc.vector.memset(hn[:, 2 + HTOK:2 + HTOK + 4], 0.0)
            nc.vector.memset(hn[:, HNW - 2:], 0.0)
            hnr = hn[:].bitcast(f32r)
            HNS = (2, 2 + HTOK + 4)        # hn write offset per half
            HR = (0, HTOK)                 # token-range start per half

            # ---------------- patch embedding (into h PSUM) ----------------
            # identity @ posbB first: start=True pending-zeros the whole
            # region and this matmul touches every byte.
            nc.tensor.matmul(out=hps[:], lhsT=ident, rhs=posbB,
                             start=True, stop=False, skip_group_check=True)
            for j in range(8):
                jq, jr = j // 4, j % 4
                rhs = xO4[:, :, jq:jq + 128, jr]
                nc.tensor.matmul(
                    out=hps[:], lhsT=pe8sb[:, j, :],
                    rhs=rhs, start=False, stop=False, skip_group_check=True)
            # zero the 3 pad tokens per sample (windows 125..127 hold junk)
            nc.vector.memset(hps_bk[:, :, 125:128], 0.0)

            # ---------------- mamba layers (two pipelined halves) -----------
            def emit_layer(l):
                w5sb, bwsb, owsb, scalsb, cbrow = lview(l)
                epsl = float(D) * EPS * (0.25 ** l)
                sq = work.tile([128, TOK], f32, tag="sq", name=f"sq_{l}")
                rstd = work.tile([1, TOK], f32, tag="rstd", name=f"rstd_{l}")
                pssum = ps.tile([1, TOK], f32, tag="pss", bufs=2, name=f"pss_{l}")
                prstd = ps.tile([128, TOK], f32, tag="prstd", name=f"prstd_{l}")
                # combined over ic, half-major: [128, half, ic, HTOK] so each
                # half's (ic, token) block is contiguous (2 PSUM banks / 4KB)
                pac = ps.tile([128, 2, 2, HTOK], f32, tag="pac", name=f"pac_{l}")
                pbc = ps.tile([128, 2, 2, HTOK], f32, tag="pbc", name=f"pbc_{l}")
                aMc = work.tile([128, 2, 2, HTOK], f32, tag="amc", bufs=2,
                                name=f"amc_{l}")
                sgc = work.tile([128, 2, 2, HTOK], f32, tag="sgc", name=f"sgc_{l}")
                abc = work.tile([128, 2, 2, HTOK], f32, tag="abc", name=f"abc_{l}")
                scc = work.tile([128, 2, 2, HTOK], f32, tag="scc", name=f"scc_{l}")
                dbc = work.tile([128, 2, 2, HTOK], f32, tag="dbc", name=f"dbc_{l}")
                ggc = work.tile([128, 2, 2, HTOK], f32, tag="ggc", name=f"ggc_{l}")

                def half2d(tile_, hh):
                    ap = tile_[:]
                    return bass.AP(tensor=ap.tensor,
                                   offset=ap.offset + hh * 2 * HTOK,
                                   ap=[list(ap.ap[0]), [1, 2 * HTOK]])

                # aM first: depends only on constants, fills ACT while the
                # previous layer's gate phase runs (bufs=2 on the amc tag)
                for hh in range(2):
                    r0 = HR[hh]
                    for ic in range(2):
                        nc.scalar.activation(
                            out=aMc[:, hh, ic, :], in_=mask01[:, r0:r0 + HTOK],
                            func=AF.Copy, scale=scalsb[:, 1, ic:ic + 1])
                for hh in range(2):
                    r0 = HR[hh]
                    nc.scalar.activation(out=sq[:, r0:r0 + HTOK],
                                         in_=hps[:, r0:r0 + HTOK], func=AF.Square)
                for hh in range(2):
                    r0 = HR[hh]
                    nc.tensor.matmul(
                        out=pssum[0:1, r0:r0 + HTOK], lhsT=onesD,
                        rhs=sq[:, r0:r0 + HTOK].bitcast(f32r),
                        start=True, stop=True, skip_group_check=True)
                for hh in range(2):
                    r0 = HR[hh]
                    nc.vector.tensor_scalar(
                        out=rstd[0:1, r0:r0 + HTOK], in0=pssum[0:1, r0:r0 + HTOK],
                        scalar1=epsl, scalar2=-0.5, op0=OP.add, op1=OP.pow)
                for hh in range(2):
                    r0 = HR[hh]
                    nc.tensor.matmul(
                        out=prstd[:, r0:r0 + HTOK], lhsT=sqrtDrow,
                        rhs=rstd[0:1, r0:r0 + HTOK].bitcast(f32r),
                        start=True, stop=True, skip_group_check=True)
                for hh in range(2):
                    r0, h0 = HR[hh], HNS[hh]
                    nc.vector.tensor_tensor(
                        out=hn[:, h0:h0 + HTOK], in0=hps[:, r0:r0 + HTOK],
                        in1=prstd[:, r0:r0 + HTOK], op=OP.mult)
                for hh in range(2):
                    h0 = HNS[hh]
                    for ic in range(2):
                        for dk in range(K5):
                            nc.tensor.matmul(
                                out=pac[:, hh, ic, :],
                                lhsT=w5sb[:, dk, ic * 128:(ic + 1) * 128],
                                rhs=hnr[:, h0 - 2 + dk:h0 - 2 + dk + HTOK],
                                start=(dk == 0), stop=False,
                                skip_group_check=True)
                        # + conv_b via rank-1 (cb row x ones row)
                        nc.tensor.matmul(
                            out=pac[:, hh, ic, :],
                            lhsT=cbrow[0:1, ic * 128:(ic + 1) * 128],
                            rhs=onesrow,
                            start=False, stop=True, skip_group_check=True)
                    for ic in range(2):
                        nc.tensor.matmul(
                            out=pbc[:, hh, ic, :],
                            lhsT=bwsb[:, ic * 128:(ic + 1) * 128],
                            rhs=hnr[:, h0:h0 + HTOK],
                            start=True, stop=True, skip_group_check=True)
                # gate phase: half-major so the two halves pipeline cleanly
                for hh in range(2):
                    for ic in range(2):
                        # silu(z) = z * sigmoid(z), z = conv + conv_b (in pac)
                        nc.scalar.activation(
                            out=sgc[:, hh, ic, :],
                            in_=pac[:, hh, ic, :], func=AF.Sigmoid)
                    nc.vector.tensor_tensor(
                        out=half2d(abc, hh), in0=half2d(pac, hh),
                        in1=half2d(sgc, hh), op=OP.mult)
                    # scan over (ic, token): the mask's zero at each sample
                    # start also resets at the ic boundary (Pool, SBUF-only)
                    nc.gpsimd.tensor_tensor_scan(
                        out=half2d(scc, hh), data0=half2d(aMc, hh),
                        data1=half2d(abc, hh), initial=0.0,
                        op0=OP.mult, op1=OP.add)
                    for ic in range(2):
                        nc.vector.tensor_scalar_mul(
                            out=dbc[:, hh, ic, :], in0=abc[:, hh, ic, :],
                            scalar1=scalsb[:, 3, ic:ic + 1])
                    for ic in range(2):
                        # g0 = gamma*beta*s + dab (Pool; SBUF operands only)
                        nc.gpsimd.scalar_tensor_tensor(
                            out=scc[:, hh, ic, :], in0=scc[:, hh, ic, :],
                            scalar=scalsb[:, 2, ic:ic + 1],
                            in1=dbc[:, hh, ic, :], op0=OP.mult, op1=OP.add)
                    # pads stay zero: hn pads are zero so pb pads are zero
                    nc.vector.tensor_tensor(
                        out=half2d(ggc, hh), in0=half2d(scc, hh),
                        in1=half2d(pbc, hh), op=OP.mult)
                    for ic in range(2):
                        # residual: h' += 2^-(l+1) * oW @ g (scale folded into oW)
                        nc.tensor.matmul(
                            out=hps[:, HR[hh]:HR[hh] + HTOK], lhsT=owsb[:, ic, :],
                            rhs=ggc[:, hh, ic, :].bitcast(f32r),
                            start=False, stop=False, skip_group_check=True)

            for l in range(NL):
                emit_layer(l)

            # ---------------- final rmsnorm ----------------
            epsf = float(D) * EPS * (0.25 ** NL)
            sqf = work.tile([128, TOK], f32, tag="sq", name="t_sqf")
            rstdf = work.tile([1, TOK], f32, tag="rstd", name="t_rstdf")
            pssumf = ps.tile([1, TOK], f32, tag="pss", bufs=2, name="t_pssf")
            prstdf = ps.tile([128, TOK], f32, tag="prstd", name="t_prstdf")
            hf = work.tile([128, TOK], bf16, tag="hf", name="t_hf")
            for hh in range(2):
                r0 = HR[hh]
                nc.scalar.activation(out=sqf[:, r0:r0 + HTOK],
                                     in_=hps[:, r0:r0 + HTOK], func=AF.Square)
            for hh in range(2):
                r0 = HR[hh]
                nc.tensor.matmul(
                    out=pssumf[0:1, r0:r0 + HTOK], lhsT=onesD,
                    rhs=sqf[:, r0:r0 + HTOK].bitcast(f32r),
                    start=True, stop=True, skip_group_check=True)
            for hh in range(2):
                r0 = HR[hh]
                nc.vector.tensor_scalar(
                    out=rstdf[0:1, r0:r0 + HTOK], in0=pssumf[0:1, r0:r0 + HTOK],
                    scalar1=epsf, scalar2=-0.5, op0=OP.add, op1=OP.pow)
            for hh in range(2):
                r0 = HR[hh]
                nc.tensor.matmul(
                    out=prstdf[:, r0:r0 + HTOK], lhsT=sqrtDrow,
                    rhs=rstdf[0:1, r0:r0 + HTOK].bitcast(f32r),
                    start=True, stop=True, skip_group_check=True)
            for hh in range(2):
                r0 = HR[hh]
                nc.vector.scalar_tensor_tensor(
                    out=hf[:, r0:r0 + HTOK], in0=hps[:, r0:r0 + HTOK],
                    scalar=normf, in1=prstdf[:, r0:r0 + HTOK],
                    op0=OP.mult, op1=OP.mult)

            # ---------------- all-gather the features (bf16) ----------------
            ccin = dram.tile([128, TOK], bf16)
            nc.scalar.dma_start(out=ccin[:], in_=hf[:])
            # inner dim padded so the gathered blocks stay stride-separated
            TOKP = TOK + 8
            ccout = dram.tile([NCORES, 128, TOKP], bf16, addr_space="Shared")
            nc.gpsimd.collective_compute(
                "AllGather", OP.bypass,
                replica_groups=[list(range(NCORES))],
                ins=[ccin[:].opt()], outs=[ccout[:, :, 0:TOK]])
            # flatT[d, b, k] (k padded to 128; pads are zero), b = r*BLOC + b4
            flatT = const.tile([128, B, 128], bf16)
            nc.scalar.dma_start(
                out=flatT[:].rearrange("p (r x) k -> p r (x k)", r=NCORES),
                in_=bass.AP(tensor=ccout[:].tensor, offset=ccout[:].offset,
                            ap=[[TOKP, 128], [128 * TOKP, NCORES], [1, TOK]]),
            )
            fap = flatT[:]
            fp0 = list(fap.ap[0])

            # ---------------- streamed output matmul ----------------
            # y[b, o] accumulated over the 125 (k, d) chunks. Stationary
            # operand = flatT columns (j, b) at offset k: output row j*32+b
            # holds sum_d flat[d, b, k+j] * wt_k[d, o]; rows 0..31 / j=0 are
            # the real batch rows, the rest M-padding. Moving operand = the
            # streamed bf16 W tile.
            pyb0 = ps.tile([128, 512], f32, tag="pac", name="t_pyb0")
            pyb1 = ps.tile([128, OSL - 512], f32, tag="pbc", name="t_pyb1")
            pybs = ((pyb0, 0, 512), (pyb1, 512, OSL - 512))
            for q in range(NQ):
                kc_n = min(WKC, NP - q * WKC)
                wtl = wring.tile([128, WKC, OSL], bf16, tag="wt", name="t_wt")
                nc.sync.dma_start(
                    out=wtl[:, 0:kc_n, :],
                    in_=bass.AP(tensor=wtT, offset=q * WKC * 128 * OSL,
                                ap=[[OSL, 128], [128 * OSL, kc_n], [1, OSL]]),
                )
                for kc in range(kc_n):
                    k = q * WKC + kc
                    lhsT = bass.AP(tensor=fap.tensor, offset=fap.offset + k,
                                   ap=[fp0, [1, 4], [128, 32]])
                    for (pt, o0, on) in pybs:
                        nc.tensor.matmul(
                            out=pt[:], lhsT=lhsT,
                            rhs=wtl[:, kc, o0:o0 + on],
                            start=(k == 0), stop=(k == NP - 1),
                            skip_group_check=True)

            yout = work.tile([32, OSL], f32, tag="yout", name="t_yout")
            nc.scalar.copy(out=yout[:, 0:512], in_=pyb0[0:32, :])
            nc.scalar.copy(out=yout[:, 512:OSL], in_=pyb1[0:32, :])
            nc.scalar.dma_start(out=y[:], in_=yout[:])

    _legalize_waits(nc)
    return nc


def _legalize_waits(nc):
    """walrus on this toolchain accepts only one sync wait per non-sequencer
    instruction. Move extra waits onto standalone InstEventSemaphore
    instructions (sequencer-level waits, multi-wait legal) placed just
    before the owning instruction on the same engine."""
    n_moved = 0
    for bb in nc.main_func.blocks:
        out = []
        for inst in bb.instructions:
            si = inst.sync_info
            tn = type(inst).__name__
            if (si is not None and len(si.on_wait) > 1
                    and tn not in ("InstEventSemaphore", "InstNoOp")):
                waits = list(si.on_wait)
                for w in waits[:-1]:
                    ev = mybir.InstNoOp(
                        name=f"lw_{inst.name}_{n_moved}", ins=[], outs=[],
                        engine=inst.engine)
                    ev.sync_info = mybir.SyncInfo(on_wait=[w], on_update=[])
                    nc.register_instruction(ev)
                    out.append(ev)
                    n_moved += 1
                inst.sync_info = mybir.SyncInfo(
                    on_wait=[waits[-1]], on_update=list(si.on_update))
            out.append(inst)
        bb.instructions = out


def _sincos_pe(n, d):
    pos = np.arange(n, dtype=np.float32)[:, None]
    sin_cols, cos_cols = (d + 1) // 2, d // 2
    denom = d / 2.0
    sin_div = np.exp(
        (-math.log(10000.0) * np.arange(sin_cols, dtype=np.float32) / denom)
    ).astype(np.float32)
    cos_div = np.exp(
        (-math.log(10000.0) * np.arange(cos_cols, dtype=np.float32) / denom)
    ).astype(np.float32)
    pe = np.zeros((n, d), dtype=np.float32)
    pe[:, 0::2] = np.sin(pos * sin_div[None, :])
    pe[:, 1::2] = np.cos(pos * cos_div[None, :])
    return pe


def _to_bf16(a):
    import ml_dtypes
    return np.asarray(a, np.float32).astype(ml_dtypes.bfloat16)


def make_in_maps(x, pe_W, pe_b, norm_w, ipa_W, ipb_W, conv_W, conv_b,
                 alpha, beta, gamma, delta, op_W, normf_w, out_W, out_b):
    f = np.float32
    x = np.asarray(x, f)
    x_pad = np.zeros((B, C, LPAD + 4), f)
    x_pad[:, :, :L] = x
    # device layout: [p2*64+c, b_loc, l] with p2=1 rows shifted by 8 along l;
    # 129 windows of 4 (the last is zero padding for the shifted matmuls)
    xcT = x_pad.transpose(1, 0, 2)                     # [c, b, lpad]
    x_dev = np.empty((2, C, B, NW * 4), f)
    x_dev[0] = xcT[:, :, 0:NW * 4]
    x_dev[1] = xcT[:, :, 8:8 + NW * 4]
    x_dev = x_dev.reshape(128, B, NW * 4)

    pw = np.asarray(pe_W, f).reshape(D, C, P)          # [d, c, p]
    t = pw.transpose(1, 2, 0)                          # [c, p, d]
    pe8 = np.ascontiguousarray(
        t.reshape(C, 2, 8, D).transpose(2, 1, 0, 3).reshape(8, 128, 128))
    pe8 = np.ascontiguousarray(pe8.transpose(1, 0, 2))  # [pp, j, d]

    posb = _sincos_pe(NP, D).T + np.asarray(pe_b, f)[:, None]   # [128, 125]
    posbB = np.zeros((128, BLOC, 128), f)
    posbB[:, :, :NP] = posb[:, None, :]
    posbB = posbB.reshape(128, TOK)

    ident = np.eye(128, dtype=f)
    sqd = np.full((128, 128), SQRTD, f)
    oner = np.ones((128, HTOK), f)
    misc = np.zeros((128, 2), f)
    misc[:, 0] = 1.0
    misc[:, 1] = np.asarray(normf_w, f)

    nw = np.asarray(norm_w, f)                          # [NL, D]
    ipa = np.asarray(ipa_W, f)                          # [NL, INNER, D]
    cw = np.asarray(conv_W, f)[:, :, 0, :]              # [NL, INNER, K5]
    w5 = (ipa.transpose(0, 2, 1)[:, None, :, :]         # [NL, 1, D, INNER]
          * cw.transpose(0, 2, 1)[:, :, None, :]        # [NL, K5, 1, INNER]
          * nw[:, None, :, None])                       # [NL, K5, D, INNER]
    bwh = np.asarray(ipb_W, f).transpose(0, 2, 1) * nw[:, :, None]  # [NL, D, INNER]
    owh = np.asarray(op_W, f).transpose(0, 2, 1).reshape(NL, 2, 128, D)

    asig = 1.0 / (1.0 + np.exp(-np.asarray(alpha, np.float64)))
    gb = np.asarray(gamma, np.float64) * np.asarray(beta, np.float64)
    p4 = np.stack([np.asarray(conv_b, f),
                   asig.astype(f),
                   gb.astype(f),
                   np.asarray(delta, f)], axis=0)       # [4, NL, INNER]

    layer_cols = []
    for l in range(NL):
        w5l = w5[l].transpose(1, 0, 2).reshape(128, K5 * INNER)   # [d,(k i)]
        bwl = bwh[l].reshape(128, INNER)                          # [d, i]
        # residual lives in PSUM as h' = h * 2^-l: fold 2^-(l+1) into op_W
        owl = np.ascontiguousarray(
            owh[l].transpose(2, 0, 1)).reshape(128, 2 * D) * (0.5 ** (l + 1))
        scl = np.ascontiguousarray(
            p4[:, l].reshape(4, 2, 128).transpose(2, 0, 1)).reshape(128, 8)
        cbr = np.zeros((128, INNER), f)
        cbr[0] = np.asarray(conv_b, f)[l]
        layer_cols.append(np.concatenate(
            [w5l, bwl, owl.astype(f), scl, cbr], axis=1))

    oW = np.asarray(out_W, f)
    pe8h = _to_bf16(pe8.reshape(128, 8 * 128))
    wpk = np.ascontiguousarray(np.concatenate(
        [posbB, ident, sqd, oner, misc] + layer_cols, axis=1))
    assert wpk.shape[1] == WCOLS, wpk.shape
    in_maps = []
    for r in range(NCORES):
        wphk = np.concatenate([
            _to_bf16(x_dev[:, r * BLOC:(r + 1) * BLOC, :].reshape(
                128, BLOC * NW * 4)),
            pe8h,
        ], axis=1)
        assert wphk.shape[1] == HCOLS, wphk.shape
        in_maps.append({
            "wph": np.ascontiguousarray(wphk),
            "wpack": wpk,
            "wt": _to_bf16(oW[r * OSL:(r + 1) * OSL].T),
        })
    return in_maps


def kernel(**inputs):
    global _PROG
    if _PROG is None:
        _PROG = build_program()
    in_maps = make_in_maps(**inputs)
    res = run_bass_kernel_spmd(_PROG, in_maps, list(range(NCORES)))
    return assemble_output([res.results[r]["y"] for r in range(NCORES)],
                           inputs["out_b"])


def assemble_output(ys, out_b=None):
    y = np.concatenate([np.asarray(yr).reshape(B, OSL) for yr in ys], axis=1)
    if out_b is not None:
        y = y + np.asarray(out_b, np.float32)[None, :]
    return y.reshape(B, C, F)


# revision 30
# speedup vs baseline: 1.0106x; 1.0106x over previous
"""Trainium2 Bass kernel for nn_CMambaSlim.

Strategy (8 NeuronCores):
  - Data-parallel trunk: each core runs the CMamba trunk (patch embed, 4
    mamba blocks, final RMSNorm) on B/8 = 4 batch samples, fp32/f32r.
  - AllGather of the flattened features (1 MB bf16) across the 8 cores.
  - Tensor-parallel output layer: core r streams rows [r*768, (r+1)*768) of
    out_W in bf16 (pre-transposed on host to [16000, 768]) and computes
    y[:, r*768:(r+1)*768]. out_b is added on the host during unsharding.

Schedule notes (CoreSim cost model):
  - All DMAs serialize on the DMA-engine device and hold the issuing
    engine's sequencer. SP's queue carries ONLY the wpack loads + the 32
    streamed weight chunks, so weight prefetch runs through the trunk and
    the AllGather. ccin/flatT/y DMAs issue from the Activation engine.
  - The residual stream h lives in PSUM: the out-projection matmuls
    accumulate straight into it (h' = h * 2^-l with the 2^-(l+1) folded
    into op_W host-side; rmsnorm is scale-invariant so only the eps
    constant needs a per-layer 4^-l).
  - The trunk is split into two independent 256-token halves (the conv
    windows are separated by a zeroed 4-column gap in hn), pipelined
    through ACT (square/sigmoid/aM), DVE (rstd/hn/u/gg/dab), Pool
    (scan/g0, SBUF-only operands), and PE.
  - ACT stays on the sigmoid table the whole trunk; rstd is computed on
    DVE as (ssum + D*eps_l)^-0.5 via AluOp.pow.
"""

import math
import os
import sys

import numpy as np

for _p in ("/opt/trn_rl_repo", "/root/.axon_site/_ro/trn_rl_repo"):
    if os.path.isdir(_p) and _p not in sys.path:
        sys.path.insert(0, _p)
        break

import concourse.bass as bass
import concourse.tile as tile
from concourse import mybir
from concourse.bass_utils import run_bass_kernel_spmd

# Model dims (hardcoded per problem spec)
B, C, L = 32, 64, 512
P, S = 16, 4
NP = 125
D = 128
INNER = 256
K5 = 5
NL = 4
F = 96
EPS = 1e-5

NCORES = 8
BLOC = B // NCORES            # 4 samples per core
OSL = (C * F) // NCORES       # 768 output cols per core
TOK = BLOC * 128              # padded token span (125 valid + 3 pad per sample)
HTOK = TOK // 2               # 256 tokens per pipelined half
HNW = 2 + HTOK + 4 + HTOK + 2  # hn with guards + inter-half gap = 520
LPAD = 520                    # x padded along L so the +8-shifted copy stays in bounds
NF = NP * D                   # 16000 contraction size
WKC = 4                       # k-chunks per weight-stream DMA
NQ = (NP + WKC - 1) // WKC    # 32 chunks (31 full + 1 partial)
WRING = 18                    # weight ring depth (chunks prefetchable)

f32 = mybir.dt.float32
f32r = mybir.dt.float32r
bf16 = mybir.dt.bfloat16
AF = mybir.ActivationFunctionType
OP = mybir.AluOpType

_PROG = None

SQRTD = math.sqrt(float(D))

# wph: bf16 embed inputs (x windows + patch-embed W), loaded first
NW = 129                                   # x windows (last is zero padding)
HOFF_PE8 = BLOC * NW * 4                   # 2064 cols of x
HCOLS = HOFF_PE8 + 8 * 128                 # + 1024 cols of patch-embed W
# wpack: fp32 consts (embed tail + one DMA per layer)
WOFF_IDN = TOK                             # 512 cols: posb broadcast to (b,k)
WOFF_SQD = WOFF_IDN + 128                  # 128 cols: identity matrix
WOFF_ONER = WOFF_SQD + 128                 # 128 cols: sqrt(D) everywhere
WOFF_MISC = WOFF_ONER + HTOK               # 256 cols: 1.0 (ones row)
WOFF_L0 = WOFF_MISC + 2                    # col 0: 1.0 (onesD), col 1: normf_w
LCOLS = K5 * INNER + INNER + 2 * D + 8 + INNER  # + 256 cols conv_b row
WCOLS = WOFF_L0 + NL * LCOLS


def build_program():
    nc = bass.Bass(num_devices=NCORES)

    wph = nc.declare_dram_parameter("wph", [128, HCOLS], bf16, isOutput=False)
    wpack = nc.declare_dram_parameter("wpack", [128, WCOLS], f32, isOutput=False)
    wt = nc.declare_dram_parameter("wt", [NF, OSL], bf16, isOutput=False)
    y = nc.declare_dram_parameter("y", [B, OSL], f32, isOutput=True)

    wtT = wt[:].tensor

    with tile.TileContext(nc) as tc:
        with (
            tc.tile_pool(name="const", bufs=1) as const,
            tc.tile_pool(name="work", bufs=1) as work,
            tc.tile_pool(name="wring", bufs=WRING) as wring,
            tc.tile_pool(name="ps", bufs=1, space="PSUM") as ps,
            tc.tile_pool(name="dram", bufs=1, space="DRAM") as dram,
        ):
            # ---------------- constant loads (embed parts, then per layer) ---
            # posbB/ident first (gates the first embed matmul), then x/pe8
            wp = const.tile([128, WCOLS], f32r)
            nc.sync.dma_start(out=wp[:, 0:WOFF_L0],
                              in_=wpack[:, 0:WOFF_L0].bitcast(f32r))
            wh = const.tile([128, HCOLS], bf16)
            nc.sync.dma_start(out=wh[:], in_=wph[:])
            for l in range(NL):
                c0 = WOFF_L0 + l * LCOLS
                nc.sync.dma_start(out=wp[:, c0:c0 + LCOLS],
                                  in_=wpack[:, c0:c0 + LCOLS].bitcast(f32r))

            xO4 = wh[:, 0:HOFF_PE8].rearrange(
                "p (b k s) -> p b k s", b=BLOC, s=4)          # [128, 4, 129, 4]
            pe8sb = wh[:, HOFF_PE8:HCOLS].rearrange("p (j d) -> p j d", j=8)
            posbB = wp[:, 0:WOFF_IDN]                          # [128, 512] (b,k)
            ident = wp[:, WOFF_IDN:WOFF_SQD]                   # I_128
            sqrtDrow = wp[0:1, WOFF_SQD:WOFF_SQD + 128]        # value sqrt(D)
            onesrow = wp[0:1, WOFF_ONER:WOFF_ONER + HTOK]      # value 1.0
            onesD = wp[:, WOFF_MISC:WOFF_MISC + 1]             # value 1.0
            normf = wp[:, WOFF_MISC + 1:WOFF_MISC + 2].bitcast(f32)

            def lview(l):
                b0 = WOFF_L0 + l * LCOLS
                w5 = wp[:, b0:b0 + K5 * INNER].rearrange(
                    "p (k i) -> p k i", k=K5)
                bw = wp[:, b0 + K5 * INNER:b0 + K5 * INNER + INNER]
                ow = wp[:, b0 + K5 * INNER + INNER:
                        b0 + K5 * INNER + INNER + 2 * D].rearrange(
                    "p (c d) -> p c d", c=2)
                scal = wp[:, b0 + LCOLS - 8 - INNER:
                          b0 + LCOLS - INNER].bitcast(f32).rearrange(
                    "p (s c) -> p s c", s=4)
                cbrow = wp[0:1, b0 + LCOLS - INNER:b0 + LCOLS]  # conv_b row
                return w5, bw, ow, scal, cbrow

            # mask01: 1 everywhere, 0 at each sample's k=0 column (scan reset)
            mask01 = const.tile([128, TOK], f32)
            nc.vector.memset(mask01[:], 1.0)
            for bq in range(BLOC):
                nc.vector.memset(mask01[:, bq * 128:bq * 128 + 1], 0.0)

            # residual stream h' lives in PSUM; out-projections accumulate
            # into it (never stopped). h' = h * 2^-l, exact via scaled op_W.
            hps = ps.tile([128, TOK], f32, tag="h", name="t_h")
            hps_bk = hps[:].rearrange("p (b k) -> p b k", b=BLOC)

            # normalized-input tile: [2 guard | half0 | 4 gap | half1 | 2 guard]
            hn = const.tile([128, HNW], f32)
            nc.vector.memset(hn[:, 0:2], 0.0)
            nc.vector.memset(hn[:, 2 + HTOK:2 + HTOK + 4], 0.0)
            nc.vector.memset(hn[:, HNW - 2:], 0.0)
            hnr = hn[:].bitcast(f32r)
            HNS = (2, 2 + HTOK + 4)        # hn write offset per half
            HR = (0, HTOK)                 # token-range start per half

            # ---------------- patch embedding (into h PSUM) ----------------
            # identity @ posbB first: start=True pending-zeros the whole
            # region and this matmul touches every byte.
            nc.tensor.matmul(out=hps[:], lhsT=ident, rhs=posbB,
                             start=True, stop=False, skip_group_check=True)
            for j in range(8):
                jq, jr = j // 4, j % 4
                rhs = xO4[:, :, jq:jq + 128, jr]
                nc.tensor.matmul(
                    out=hps[:], lhsT=pe8sb[:, j, :],
                    rhs=rhs, start=False, stop=False, skip_group_check=True)
            # zero the 3 pad tokens per sample (windows 125..127 hold junk)
            nc.vector.memset(hps_bk[:, :, 125:128], 0.0)

            # ---------------- mamba layers (two pipelined halves) -----------
            def emit_layer(l):
                w5sb, bwsb, owsb, scalsb, cbrow = lview(l)
                epsl = float(D) * EPS * (0.25 ** l)
                sq = work.tile([128, TOK], f32, tag="sq", name=f"sq_{l}")
                rstd = work.tile([1, TOK], f32, tag="rstd", name=f"rstd_{l}")
                pssum = ps.tile([1, TOK], f32, tag="pss", bufs=2, name=f"pss_{l}")
                prstd = ps.tile([128, TOK], f32, tag="prstd", name=f"prstd_{l}")
                # combined over ic, half-major: [128, half, ic, HTOK] so each
                # half's (ic, token) block is contiguous (2 PSUM banks / 4KB)
                pac = ps.tile([128, 2, 2, HTOK], f32, tag="pac", name=f"pac_{l}")
                pbc = ps.tile([128, 2, 2, HTOK], f32, tag="pbc", name=f"pbc_{l}")
                aMc = work.tile([128, 2, 2, HTOK], f32, tag="amc", bufs=2,
                                name=f"amc_{l}")
                sgc = work.tile([128, 2, 2, HTOK], f32, tag="sgc", name=f"sgc_{l}")
                abc = work.tile([128, 2, 2, HTOK], f32, tag="abc", name=f"abc_{l}")
                scc = work.tile([128, 2, 2, HTOK], f32, tag="scc", name=f"scc_{l}")
                dbc = work.tile([128, 2, 2, HTOK], f32, tag="dbc", name=f"dbc_{l}")
                ggc = work.tile([128, 2, 2, HTOK], f32, tag="ggc", name=f"ggc_{l}")

                def half2d(tile_, hh):
                    ap = tile_[:]
                    return bass.AP(tensor=ap.tensor,
                                   offset=ap.offset + hh * 2 * HTOK,
                                   ap=[list(ap.ap[0]), [1, 2 * HTOK]])

                # aM first: depends only on constants, fills ACT while the
                # previous layer's gate phase runs (bufs=2 on the amc tag)
                for hh in range(2):
                    r0 = HR[hh]
                    for ic in range(2):
                        nc.scalar.activation(
                            out=aMc[:, hh, ic, :], in_=mask01[:, r0:r0 + HTOK],
                            func=AF.Copy, scale=scalsb[:, 1, ic:ic + 1])
                for hh in range(2):
                    r0 = HR[hh]
                    nc.scalar.activation(out=sq[:, r0:r0 + HTOK],
                                         in_=hps[:, r0:r0 + HTOK], func=AF.Square)
                for hh in range(2):
                    r0 = HR[hh]
                    nc.tensor.matmul(
                        out=pssum[0:1, r0:r0 + HTOK], lhsT=onesD,
                        rhs=sq[:, r0:r0 + HTOK].bitcast(f32r),
                        start=True, stop=True, skip_group_check=True)
                for hh in range(2):
                    r0 = HR[hh]
                    nc.vector.tensor_scalar(
                        out=rstd[0:1, r0:r0 + HTOK], in0=pssum[0:1, r0:r0 + HTOK],
                        scalar1=epsl, scalar2=-0.5, op0=OP.add, op1=OP.pow)
                for hh in range(2):
                    r0 = HR[hh]
                    nc.tensor.matmul(
                        out=prstd[:, r0:r0 + HTOK], lhsT=sqrtDrow,
                        rhs=rstd[0:1, r0:r0 + HTOK].bitcast(f32r),
                        start=True, stop=True, skip_group_check=True)
                for hh in range(2):
                    r0, h0 = HR[hh], HNS[hh]
                    nc.vector.tensor_tensor(
                        out=hn[:, h0:h0 + HTOK], in0=hps[:, r0:r0 + HTOK],
                        in1=prstd[:, r0:r0 + HTOK], op=OP.mult)
                for hh in range(2):
                    h0 = HNS[hh]
                    for ic in range(2):
                        for dk in range(K5):
                            nc.tensor.matmul(
                                out=pac[:, hh, ic, :],
                                lhsT=w5sb[:, dk, ic * 128:(ic + 1) * 128],
                                rhs=hnr[:, h0 - 2 + dk:h0 - 2 + dk + HTOK],
                                start=(dk == 0), stop=False,
                                skip_group_check=True)
                        # + conv_b via rank-1 (cb row x ones row)
                        nc.tensor.matmul(
                            out=pac[:, hh, ic, :],
                            lhsT=cbrow[0:1, ic * 128:(ic + 1) * 128],
                            rhs=onesrow,
                            start=False, stop=True, skip_group_check=True)
                    for ic in range(2):
                        nc.tensor.matmul(
                            out=pbc[:, hh, ic, :],
                            lhsT=bwsb[:, ic * 128:(ic + 1) * 128],
                            rhs=hnr[:, h0:h0 + HTOK],
                            start=True, stop=True, skip_group_check=True)
                # gate phase: half-major so the two halves pipeline cleanly
                for hh in range(2):
                    for ic in range(2):
                        # silu(z) = z * sigmoid(z), z = conv + conv_b (in pac)
                        nc.scalar.activation(
                            out=sgc[:, hh, ic, :],
                            in_=pac[:, hh, ic, :], func=AF.Sigmoid)
                    for ic in range(2):
                        nc.vector.tensor_tensor(
                            out=abc[:, hh, ic, :], in0=pac[:, hh, ic, :],
                            in1=sgc[:, hh, ic, :], op=OP.mult)
                    for ic in range(2):
                        # scan: state = aM*state + u (Pool; SBUF operands only)
                        nc.gpsimd.tensor_tensor_scan(
                            out=scc[:, hh, ic, :], data0=aMc[:, hh, ic, :],
                            data1=abc[:, hh, ic, :], initial=0.0,
                            op0=OP.mult, op1=OP.add)
                    for ic in range(2):
                        nc.vector.tensor_scalar_mul(
                            out=dbc[:, hh, ic, :], in0=abc[:, hh, ic, :],
                            scalar1=scalsb[:, 3, ic:ic + 1])
                    for ic in range(2):
                        # g0 = gamma*beta*s + dab (Pool; SBUF operands only)
                        nc.gpsimd.scalar_tensor_tensor(
                            out=scc[:, hh, ic, :], in0=scc[:, hh, ic, :],
                            scalar=scalsb[:, 2, ic:ic + 1],
                            in1=dbc[:, hh, ic, :], op0=OP.mult, op1=OP.add)
                    for ic in range(2):
                        # pads stay zero: hn pads are zero so pb pads are zero
                        nc.vector.tensor_tensor(
                            out=ggc[:, hh, ic, :], in0=scc[:, hh, ic, :],
                            in1=pbc[:, hh, ic, :], op=OP.mult)
                    for ic in range(2):
                        # residual: h' += 2^-(l+1) * oW @ g (scale folded into oW)
                        nc.tensor.matmul(
                            out=hps[:, HR[hh]:HR[hh] + HTOK], lhsT=owsb[:, ic, :],
                            rhs=ggc[:, hh, ic, :].bitcast(f32r),
                            start=False, stop=False, skip_group_check=True)

            for l in range(NL):
                emit_layer(l)

            # ---------------- final rmsnorm ----------------
            epsf = float(D) * EPS * (0.25 ** NL)
            sqf = work.tile([128, TOK], f32, tag="sq", name="t_sqf")
            rstdf = work.tile([1, TOK], f32, tag="rstd", name="t_rstdf")
            pssumf = ps.tile([1, TOK], f32, tag="pss", bufs=2, name="t_pssf")
            prstdf = ps.tile([128, TOK], f32, tag="prstd", name="t_prstdf")
            hf = work.tile([128, TOK], bf16, tag="hf", name="t_hf")
            for hh in range(2):
                r0 = HR[hh]
                nc.scalar.activation(out=sqf[:, r0:r0 + HTOK],
                                     in_=hps[:, r0:r0 + HTOK], func=AF.Square)
            for hh in range(2):
                r0 = HR[hh]
                nc.tensor.matmul(
                    out=pssumf[0:1, r0:r0 + HTOK], lhsT=onesD,
                    rhs=sqf[:, r0:r0 + HTOK].bitcast(f32r),
                    start=True, stop=True, skip_group_check=True)
            for hh in range(2):
                r0 = HR[hh]
                nc.vector.tensor_scalar(
                    out=rstdf[0:1, r0:r0 + HTOK], in0=pssumf[0:1, r0:r0 + HTOK],
                    scalar1=epsf, scalar2=-0.5, op0=OP.add, op1=OP.pow)
            for hh in range(2):
                r0 = HR[hh]
                nc.tensor.matmul(
                    out=prstdf[:, r0:r0 + HTOK], lhsT=sqrtDrow,
                    rhs=rstdf[0:1, r0:r0 + HTOK].bitcast(f32r),
                    start=True, stop=True, skip_group_check=True)
            for hh in range(2):
                r0 = HR[hh]
                nc.vector.scalar_tensor_tensor(
                    out=hf[:, r0:r0 + HTOK], in0=hps[:, r0:r0 + HTOK],
                    scalar=normf, in1=prstdf[:, r0:r0 + HTOK],
                    op0=OP.mult, op1=OP.mult)

            # ---------------- all-gather the features (bf16) ----------------
            ccin = dram.tile([128, TOK], bf16)
            nc.scalar.dma_start(out=ccin[:], in_=hf[:])
            # inner dim padded so the gathered blocks stay stride-separated
            TOKP = TOK + 8
            ccout = dram.tile([NCORES, 128, TOKP], bf16, addr_space="Shared")
            nc.gpsimd.collective_compute(
                "AllGather", OP.bypass,
                replica_groups=[list(range(NCORES))],
                ins=[ccin[:].opt()], outs=[ccout[:, :, 0:TOK]])
            # flatT[d, b, k] (k padded to 128; pads are zero), b = r*BLOC + b4
            flatT = const.tile([128, B, 128], bf16)
            nc.scalar.dma_start(
                out=flatT[:].rearrange("p (r x) k -> p r (x k)", r=NCORES),
                in_=bass.AP(tensor=ccout[:].tensor, offset=ccout[:].offset,
                            ap=[[TOKP, 128], [128 * TOKP, NCORES], [1, TOK]]),
            )
            fap = flatT[:]
            fp0 = list(fap.ap[0])

            # ---------------- streamed output matmul ----------------
            # y[b, o] accumulated over the 125 (k, d) chunks. Stationary
            # operand = flatT columns (j, b) at offset k: output row j*32+b
            # holds sum_d flat[d, b, k+j] * wt_k[d, o]; rows 0..31 / j=0 are
            # the real batch rows, the rest M-padding. Moving operand = the
            # streamed bf16 W tile.
            pyb0 = ps.tile([128, 512], f32, tag="pac", name="t_pyb0")
            pyb1 = ps.tile([128, OSL - 512], f32, tag="pbc", name="t_pyb1")
            pybs = ((pyb0, 0, 512), (pyb1, 512, OSL - 512))
            for q in range(NQ):
                kc_n = min(WKC, NP - q * WKC)
                wtl = wring.tile([128, WKC, OSL], bf16, tag="wt", name="t_wt")
                nc.sync.dma_start(
                    out=wtl[:, 0:kc_n, :],
                    in_=bass.AP(tensor=wtT, offset=q * WKC * 128 * OSL,
                                ap=[[OSL, 128], [128 * OSL, kc_n], [1, OSL]]),
                )
                for kc in range(kc_n):
                    k = q * WKC + kc
                    lhsT = bass.AP(tensor=fap.tensor, offset=fap.offset + k,
                                   ap=[fp0, [1, 4], [128, 32]])
                    for (pt, o0, on) in pybs:
                        nc.tensor.matmul(
                            out=pt[:], lhsT=lhsT,
                            rhs=wtl[:, kc, o0:o0 + on],
                            start=(k == 0), stop=(k == NP - 1),
                            skip_group_check=True)

            yout = work.tile([32, OSL], f32, tag="yout", name="t_yout")
            nc.scalar.copy(out=yout[:, 0:512], in_=pyb0[0:32, :])
            nc.scalar.copy(out=yout[:, 512:OSL], in_=pyb1[0:32, :])
            nc.scalar.dma_start(out=y[:], in_=yout[:])

    _legalize_waits(nc)
    return nc


def _legalize_waits(nc):
    """walrus on this toolchain accepts only one sync wait per non-sequencer
    instruction. Move extra waits onto standalone InstEventSemaphore
    instructions (sequencer-level waits, multi-wait legal) placed just
    before the owning instruction on the same engine."""
    n_moved = 0
    for bb in nc.main_func.blocks:
        out = []
        for inst in bb.instructions:
            si = inst.sync_info
            tn = type(inst).__name__
            if (si is not None and len(si.on_wait) > 1
                    and tn not in ("InstEventSemaphore", "InstNoOp")):
                waits = list(si.on_wait)
                for w in waits[:-1]:
                    ev = mybir.InstNoOp(
                        name=f"lw_{inst.name}_{n_moved}", ins=[], outs=[],
                        engine=inst.engine)
                    ev.sync_info = mybir.SyncInfo(on_wait=[w], on_update=[])
                    nc.register_instruction(ev)
                    out.append(ev)
                    n_moved += 1
                inst.sync_info = mybir.SyncInfo(
                    on_wait=[waits[-1]], on_update=list(si.on_update))
            out.append(inst)
        bb.instructions = out


def _sincos_pe(n, d):
    pos = np.arange(n, dtype=np.float32)[:, None]
    sin_cols, cos_cols = (d + 1) // 2, d // 2
    denom = d / 2.0
    sin_div = np.exp(
        (-math.log(10000.0) * np.arange(sin_cols, dtype=np.float32) / denom)
    ).astype(np.float32)
    cos_div = np.exp(
        (-math.log(10000.0) * np.arange(cos_cols, dtype=np.float32) / denom)
    ).astype(np.float32)
    pe = np.zeros((n, d), dtype=np.float32)
    pe[:, 0::2] = np.sin(pos * sin_div[None, :])
    pe[:, 1::2] = np.cos(pos * cos_div[None, :])
    return pe


def _to_bf16(a):
    import ml_dtypes
    return np.asarray(a, np.float32).astype(ml_dtypes.bfloat16)


def make_in_maps(x, pe_W, pe_b, norm_w, ipa_W, ipb_W, conv_W, conv_b,
                 alpha, beta, gamma, delta, op_W, normf_w, out_W, out_b):
    f = np.float32
    x = np.asarray(x, f)
    x_pad = np.zeros((B, C, LPAD + 4), f)
    x_pad[:, :, :L] = x
    # device layout: [p2*64+c, b_loc, l] with p2=1 rows shifted by 8 along l;
    # 129 windows of 4 (the last is zero padding for the shifted matmuls)
    xcT = x_pad.transpose(1, 0, 2)                     # [c, b, lpad]
    x_dev = np.empty((2, C, B, NW * 4), f)
    x_dev[0] = xcT[:, :, 0:NW * 4]
    x_dev[1] = xcT[:, :, 8:8 + NW * 4]
    x_dev = x_dev.reshape(128, B, NW * 4)

    pw = np.asarray(pe_W, f).reshape(D, C, P)          # [d, c, p]
    t = pw.transpose(1, 2, 0)                          # [c, p, d]
    pe8 = np.ascontiguousarray(
        t.reshape(C, 2, 8, D).transpose(2, 1, 0, 3).reshape(8, 128, 128))
    pe8 = np.ascontiguousarray(pe8.transpose(1, 0, 2))  # [pp, j, d]

    posb = _sincos_pe(NP, D).T + np.asarray(pe_b, f)[:, None]   # [128, 125]
    posbB = np.zeros((128, BLOC, 128), f)
    posbB[:, :, :NP] = posb[:, None, :]
    posbB = posbB.reshape(128, TOK)

    ident = np.eye(128, dtype=f)
    sqd = np.full((128, 128), SQRTD, f)
    oner = np.ones((128, HTOK), f)
    misc = np.zeros((128, 2), f)
    misc[:, 0] = 1.0
    misc[:, 1] = np.asarray(normf_w, f)

    nw = np.asarray(norm_w, f)                          # [NL, D]
    ipa = np.asarray(ipa_W, f)                          # [NL, INNER, D]
    cw = np.asarray(conv_W, f)[:, :, 0, :]              # [NL, INNER, K5]
    w5 = (ipa.transpose(0, 2, 1)[:, None, :, :]         # [NL, 1, D, INNER]
          * cw.transpose(0, 2, 1)[:, :, None, :]        # [NL, K5, 1, INNER]
          * nw[:, None, :, None])                       # [NL, K5, D, INNER]
    bwh = np.asarray(ipb_W, f).transpose(0, 2, 1) * nw[:, :, None]  # [NL, D, INNER]
    owh = np.asarray(op_W, f).transpose(0, 2, 1).reshape(NL, 2, 128, D)

    asig = 1.0 / (1.0 + np.exp(-np.asarray(alpha, np.float64)))
    gb = np.asarray(gamma, np.float64) * np.asarray(beta, np.float64)
    p4 = np.stack([np.asarray(conv_b, f),
                   asig.astype(f),
                   gb.astype(f),
                   np.asarray(delta, f)], axis=0)       # [4, NL, INNER]

    layer_cols = []
    for l in range(NL):
        w5l = w5[l].transpose(1, 0, 2).reshape(128, K5 * INNER)   # [d,(k i)]
        bwl = bwh[l].reshape(128, INNER)                          # [d, i]
        # residual lives in PSUM as h' = h * 2^-l: fold 2^-(l+1) into op_W
        owl = np.ascontiguousarray(
            owh[l].transpose(2, 0, 1)).reshape(128, 2 * D) * (0.5 ** (l + 1))
        scl = np.ascontiguousarray(
            p4[:, l].reshape(4, 2, 128).transpose(2, 0, 1)).reshape(128, 8)
        cbr = np.zeros((128, INNER), f)
        cbr[0] = np.asarray(conv_b, f)[l]
        layer_cols.append(np.concatenate(
            [w5l, bwl, owl.astype(f), scl, cbr], axis=1))

    oW = np.asarray(out_W, f)
    pe8h = _to_bf16(pe8.reshape(128, 8 * 128))
    wpk = np.ascontiguousarray(np.concatenate(
        [posbB, ident, sqd, oner, misc] + layer_cols, axis=1))
    assert wpk.shape[1] == WCOLS, wpk.shape
    in_maps = []
    for r in range(NCORES):
        wphk = np.concatenate([
            _to_bf16(x_dev[:, r * BLOC:(r + 1) * BLOC, :].reshape(
                128, BLOC * NW * 4)),
            pe8h,
        ], axis=1)
        assert wphk.shape[1] == HCOLS, wphk.shape
        in_maps.append({
            "wph": np.ascontiguousarray(wphk),
            "wpack": wpk,
            "wt": _to_bf16(oW[r * OSL:(r + 1) * OSL].T),
        })
    return in_maps


def kernel(**inputs):
    global _PROG
    if _PROG is None:
        _PROG = build_program()
    in_maps = make_in_maps(**inputs)
    res = run_bass_kernel_spmd(_PROG, in_maps, list(range(NCORES)))
    return assemble_output([res.results[r]["y"] for r in range(NCORES)],
                           inputs["out_b"])


def assemble_output(ys, out_b=None):
    y = np.concatenate([np.asarray(yr).reshape(B, OSL) for yr in ys], axis=1)
    if out_b is not None:
        y = y + np.asarray(out_b, np.float32)[None, :]
    return y.reshape(B, C, F)


# revision 31
# speedup vs baseline: 1.0243x; 1.0135x over previous
"""Trainium2 Bass kernel for nn_CMambaSlim.

Strategy (8 NeuronCores):
  - Data-parallel trunk: each core runs the CMamba trunk (patch embed, 4
    mamba blocks, final RMSNorm) on B/8 = 4 batch samples, fp32/f32r.
  - AllGather of the flattened features (1 MB bf16) across the 8 cores.
  - Tensor-parallel output layer: core r streams rows [r*768, (r+1)*768) of
    out_W in bf16 (pre-transposed on host to [16000, 768]) and computes
    y[:, r*768:(r+1)*768]. out_b is added on the host during unsharding.

Schedule notes (CoreSim cost model):
  - All DMAs serialize on the DMA-engine device and hold the issuing
    engine's sequencer. SP's queue carries ONLY the wpack loads + the 32
    streamed weight chunks, so weight prefetch runs through the trunk and
    the AllGather. ccin/flatT/y DMAs issue from the Activation engine.
  - The residual stream h lives in PSUM: the out-projection matmuls
    accumulate straight into it (h' = h * 2^-l with the 2^-(l+1) folded
    into op_W host-side; rmsnorm is scale-invariant so only the eps
    constant needs a per-layer 4^-l).
  - The trunk is split into two independent 256-token halves (the conv
    windows are separated by a zeroed 4-column gap in hn), pipelined
    through ACT (square/sigmoid/aM), DVE (rstd/hn/u/gg/dab), Pool
    (scan/g0, SBUF-only operands), and PE.
  - ACT stays on the sigmoid table the whole trunk; rstd is computed on
    DVE as (ssum + D*eps_l)^-0.5 via AluOp.pow.
"""

import math
import os
import sys

import numpy as np

for _p in ("/opt/trn_rl_repo", "/root/.axon_site/_ro/trn_rl_repo"):
    if os.path.isdir(_p) and _p not in sys.path:
        sys.path.insert(0, _p)
        break

import concourse.bass as bass
import concourse.tile as tile
from concourse import mybir
from concourse.bass_utils import run_bass_kernel_spmd

# Model dims (hardcoded per problem spec)
B, C, L = 32, 64, 512
P, S = 16, 4
NP = 125
D = 128
INNER = 256
K5 = 5
NL = 4
F = 96
EPS = 1e-5

NCORES = 8
BLOC = B // NCORES            # 4 samples per core
OSL = (C * F) // NCORES       # 768 output cols per core
TOK = BLOC * 128              # padded token span (125 valid + 3 pad per sample)
HTOK = TOK // 2               # 256 tokens per pipelined half
HNW = 2 + HTOK + 4 + HTOK + 2  # hn with guards + inter-half gap = 520
LPAD = 520                    # x padded along L so the +8-shifted copy stays in bounds
NF = NP * D                   # 16000 contraction size
WKC = 4                       # k-chunks per weight-stream DMA
NQ = (NP + WKC - 1) // WKC    # 32 chunks (31 full + 1 partial)
WRING = 18                    # weight ring depth (chunks prefetchable)

f32 = mybir.dt.float32
f32r = mybir.dt.float32r
bf16 = mybir.dt.bfloat16
AF = mybir.ActivationFunctionType
OP = mybir.AluOpType

_PROG = None

SQRTD = math.sqrt(float(D))

# wph: bf16 embed inputs (x windows + patch-embed W), loaded first
NW = 129                                   # x windows (last is zero padding)
HOFF_PE8 = BLOC * NW * 4                   # 2064 cols of x
HCOLS = HOFF_PE8 + 8 * 128                 # + 1024 cols of patch-embed W
# wpack: fp32 consts (embed tail + one DMA per layer)
WOFF_IDN = TOK                             # 512 cols: posb broadcast to (b,k)
WOFF_SQD = WOFF_IDN + 128                  # 128 cols: identity matrix
WOFF_ONER = WOFF_SQD + 128                 # 128 cols: sqrt(D) everywhere
WOFF_MISC = WOFF_ONER + HTOK               # 256 cols: 1.0 (ones row)
WOFF_L0 = WOFF_MISC + 2                    # col 0: 1.0 (onesD), col 1: normf_w
LCOLS = K5 * INNER + INNER + 2 * D + 8 + INNER  # + 256 cols conv_b row
WCOLS = WOFF_L0 + NL * LCOLS


def build_program():
    nc = bass.Bass(num_devices=NCORES)

    wph = nc.declare_dram_parameter("wph", [128, HCOLS], bf16, isOutput=False)
    wpack = nc.declare_dram_parameter("wpack", [128, WCOLS], f32, isOutput=False)
    wt = nc.declare_dram_parameter("wt", [NF, OSL], bf16, isOutput=False)
    y = nc.declare_dram_parameter("y", [B, OSL], f32, isOutput=True)

    wtT = wt[:].tensor

    with tile.TileContext(nc) as tc:
        with (
            tc.tile_pool(name="const", bufs=1) as const,
            tc.tile_pool(name="work", bufs=1) as work,
            tc.tile_pool(name="wring", bufs=WRING) as wring,
            tc.tile_pool(name="ps", bufs=1, space="PSUM") as ps,
            tc.tile_pool(name="dram", bufs=1, space="DRAM") as dram,
        ):
            # ---------------- constant loads (embed parts, then per layer) ---
            # posbB/ident first (gates the first embed matmul), then x/pe8
            wp = const.tile([128, WCOLS], f32r)
            nc.sync.dma_start(out=wp[:, 0:WOFF_L0],
                              in_=wpack[:, 0:WOFF_L0].bitcast(f32r))
            wh = const.tile([128, HCOLS], bf16)
            nc.sync.dma_start(out=wh[:], in_=wph[:])
            for l in range(NL):
                c0 = WOFF_L0 + l * LCOLS
                nc.sync.dma_start(out=wp[:, c0:c0 + LCOLS],
                                  in_=wpack[:, c0:c0 + LCOLS].bitcast(f32r))

            xO4 = wh[:, 0:HOFF_PE8].rearrange(
                "p (b k s) -> p b k s", b=BLOC, s=4)          # [128, 4, 129, 4]
            pe8sb = wh[:, HOFF_PE8:HCOLS].rearrange("p (j d) -> p j d", j=8)
            posbB = wp[:, 0:WOFF_IDN]                          # [128, 512] (b,k)
            ident = wp[:, WOFF_IDN:WOFF_SQD]                   # I_128
            sqrtDrow = wp[0:1, WOFF_SQD:WOFF_SQD + 128]        # value sqrt(D)
            onesrow = wp[0:1, WOFF_ONER:WOFF_ONER + HTOK]      # value 1.0
            onesD = wp[:, WOFF_MISC:WOFF_MISC + 1]             # value 1.0
            normf = wp[:, WOFF_MISC + 1:WOFF_MISC + 2].bitcast(f32)

            def lview(l):
                b0 = WOFF_L0 + l * LCOLS
                w5 = wp[:, b0:b0 + K5 * INNER].rearrange(
                    "p (k i) -> p k i", k=K5)
                bw = wp[:, b0 + K5 * INNER:b0 + K5 * INNER + INNER]
                ow = wp[:, b0 + K5 * INNER + INNER:
                        b0 + K5 * INNER + INNER + 2 * D].rearrange(
                    "p (c d) -> p c d", c=2)
                scal = wp[:, b0 + LCOLS - 8 - INNER:
                          b0 + LCOLS - INNER].bitcast(f32).rearrange(
                    "p (s c) -> p s c", s=4)
                cbrow = wp[0:1, b0 + LCOLS - INNER:b0 + LCOLS]  # conv_b row
                return w5, bw, ow, scal, cbrow

            # mask01: 1 everywhere, 0 at each sample's k=0 column (scan reset)
            mask01 = const.tile([128, TOK], f32)
            nc.vector.memset(mask01[:], 1.0)
            for bq in range(BLOC):
                nc.vector.memset(mask01[:, bq * 128:bq * 128 + 1], 0.0)

            # residual stream h' lives in PSUM; out-projections accumulate
            # into it (never stopped). h' = h * 2^-l, exact via scaled op_W.
            hps = ps.tile([128, TOK], f32, tag="h", name="t_h")
            hps_bk = hps[:].rearrange("p (b k) -> p b k", b=BLOC)

            # normalized-input tile: [2 guard | half0 | 4 gap | half1 | 2 guard]
            hn = const.tile([128, HNW], f32)
            nc.vector.memset(hn[:, 0:2], 0.0)
            nc.vector.memset(hn[:, 2 + HTOK:2 + HTOK + 4], 0.0)
            nc.vector.memset(hn[:, HNW - 2:], 0.0)
            hnr = hn[:].bitcast(f32r)
            HNS = (2, 2 + HTOK + 4)        # hn write offset per half
            HR = (0, HTOK)                 # token-range start per half

            # ---------------- patch embedding (into h PSUM) ----------------
            # identity @ posbB first: start=True pending-zeros the whole
            # region and this matmul touches every byte.
            nc.tensor.matmul(out=hps[:], lhsT=ident, rhs=posbB,
                             start=True, stop=False, skip_group_check=True)
            for j in range(8):
                jq, jr = j // 4, j % 4
                rhs = xO4[:, :, jq:jq + 128, jr]
                nc.tensor.matmul(
                    out=hps[:], lhsT=pe8sb[:, j, :],
                    rhs=rhs, start=False, stop=False, skip_group_check=True)
            # zero the 3 pad tokens per sample (windows 125..127 hold junk)
            nc.vector.memset(hps_bk[:, :, 125:128], 0.0)

            # ---------------- mamba layers (two pipelined halves) -----------
            def emit_layer(l):
                w5sb, bwsb, owsb, scalsb, cbrow = lview(l)
                epsl = float(D) * EPS * (0.25 ** l)
                sq = work.tile([128, TOK], f32, tag="sq", name=f"sq_{l}")
                rstd = work.tile([1, TOK], f32, tag="rstd", name=f"rstd_{l}")
                pssum = ps.tile([1, TOK], f32, tag="pss", bufs=2, name=f"pss_{l}")
                prstd = ps.tile([128, TOK], f32, tag="prstd", name=f"prstd_{l}")
                # combined over ic, half-major: [128, half, ic, HTOK] so each
                # half's (ic, token) block is contiguous (2 PSUM banks / 4KB)
                pac = ps.tile([128, 2, 2, HTOK], f32, tag="pac", name=f"pac_{l}")
                pbc = ps.tile([128, 2, 2, HTOK], f32, tag="pbc", name=f"pbc_{l}")
                aMc = work.tile([128, 2, 2, HTOK], f32, tag="amc", bufs=2,
                                name=f"amc_{l}")
                sgc = work.tile([128, 2, 2, HTOK], f32, tag="sgc", name=f"sgc_{l}")
                abc = work.tile([128, 2, 2, HTOK], f32, tag="abc", name=f"abc_{l}")
                scc = work.tile([128, 2, 2, HTOK], f32, tag="scc", name=f"scc_{l}")
                dbc = work.tile([128, 2, 2, HTOK], f32, tag="dbc", name=f"dbc_{l}")
                ggc = work.tile([128, 2, 2, HTOK], f32, tag="ggc", name=f"ggc_{l}")

                def half2d(tile_, hh):
                    ap = tile_[:]
                    return bass.AP(tensor=ap.tensor,
                                   offset=ap.offset + hh * 2 * HTOK,
                                   ap=[list(ap.ap[0]), [1, 2 * HTOK]])

                # aM first: depends only on constants, fills ACT while the
                # previous layer's gate phase runs (bufs=2 on the amc tag)
                for hh in range(2):
                    r0 = HR[hh]
                    for ic in range(2):
                        nc.scalar.activation(
                            out=aMc[:, hh, ic, :], in_=mask01[:, r0:r0 + HTOK],
                            func=AF.Copy, scale=scalsb[:, 1, ic:ic + 1])
                for hh in range(2):
                    r0 = HR[hh]
                    nc.scalar.activation(out=sq[:, r0:r0 + HTOK],
                                         in_=hps[:, r0:r0 + HTOK], func=AF.Square)
                for hh in range(2):
                    r0 = HR[hh]
                    nc.tensor.matmul(
                        out=pssum[0:1, r0:r0 + HTOK], lhsT=onesD,
                        rhs=sq[:, r0:r0 + HTOK].bitcast(f32r),
                        start=True, stop=True, skip_group_check=True)
                for hh in range(2):
                    r0 = HR[hh]
                    nc.vector.tensor_scalar(
                        out=rstd[0:1, r0:r0 + HTOK], in0=pssum[0:1, r0:r0 + HTOK],
                        scalar1=epsl, scalar2=-0.5, op0=OP.add, op1=OP.pow)
                for hh in range(2):
                    r0 = HR[hh]
                    nc.tensor.matmul(
                        out=prstd[:, r0:r0 + HTOK], lhsT=sqrtDrow,
                        rhs=rstd[0:1, r0:r0 + HTOK].bitcast(f32r),
                        start=True, stop=True, skip_group_check=True)
                for hh in range(2):
                    r0, h0 = HR[hh], HNS[hh]
                    nc.vector.tensor_tensor(
                        out=hn[:, h0:h0 + HTOK], in0=hps[:, r0:r0 + HTOK],
                        in1=prstd[:, r0:r0 + HTOK], op=OP.mult)
                for hh in range(2):
                    h0 = HNS[hh]
                    for ic in range(2):
                        for dk in range(K5):
                            nc.tensor.matmul(
                                out=pac[:, hh, ic, :],
                                lhsT=w5sb[:, dk, ic * 128:(ic + 1) * 128],
                                rhs=hnr[:, h0 - 2 + dk:h0 - 2 + dk + HTOK],
                                start=(dk == 0), stop=(dk == K5 - 1),
                                skip_group_check=True)
                    for ic in range(2):
                        nc.tensor.matmul(
                            out=pbc[:, hh, ic, :],
                            lhsT=bwsb[:, ic * 128:(ic + 1) * 128],
                            rhs=hnr[:, h0:h0 + HTOK],
                            start=True, stop=True, skip_group_check=True)
                # gate phase: half-major so the two halves pipeline cleanly
                for hh in range(2):
                    for ic in range(2):
                        # silu(z) = z * sigmoid(z), z = conv + conv_b
                        nc.scalar.activation(
                            out=sgc[:, hh, ic, :], in_=pac[:, hh, ic, :],
                            func=AF.Sigmoid, bias=scalsb[:, 0, ic:ic + 1],
                            scale=1.0)
                    for ic in range(2):
                        nc.vector.scalar_tensor_tensor(
                            out=abc[:, hh, ic, :], in0=pac[:, hh, ic, :],
                            scalar=scalsb[:, 0, ic:ic + 1],
                            in1=sgc[:, hh, ic, :], op0=OP.add, op1=OP.mult)
                    for ic in range(2):
                        # scan: state = aM*state + u (Pool; SBUF operands only)
                        nc.gpsimd.tensor_tensor_scan(
                            out=scc[:, hh, ic, :], data0=aMc[:, hh, ic, :],
                            data1=abc[:, hh, ic, :], initial=0.0,
                            op0=OP.mult, op1=OP.add)
                    for ic in range(2):
                        nc.vector.tensor_scalar_mul(
                            out=dbc[:, hh, ic, :], in0=abc[:, hh, ic, :],
                            scalar1=scalsb[:, 3, ic:ic + 1])
                    for ic in range(2):
                        # g0 = gamma*beta*s + dab (Pool; SBUF operands only)
                        nc.gpsimd.scalar_tensor_tensor(
                            out=scc[:, hh, ic, :], in0=scc[:, hh, ic, :],
                            scalar=scalsb[:, 2, ic:ic + 1],
                            in1=dbc[:, hh, ic, :], op0=OP.mult, op1=OP.add)
                    for ic in range(2):
                        # pads stay zero: hn pads are zero so pb pads are zero
                        nc.vector.tensor_tensor(
                            out=ggc[:, hh, ic, :], in0=scc[:, hh, ic, :],
                            in1=pbc[:, hh, ic, :], op=OP.mult)
                    for ic in range(2):
                        # residual: h' += 2^-(l+1) * oW @ g (scale folded into oW)
                        nc.tensor.matmul(
                            out=hps[:, HR[hh]:HR[hh] + HTOK], lhsT=owsb[:, ic, :],
                            rhs=ggc[:, hh, ic, :].bitcast(f32r),
                            start=False, stop=False, skip_group_check=True)

            for l in range(NL):
                emit_layer(l)

            # ---------------- final rmsnorm ----------------
            epsf = float(D) * EPS * (0.25 ** NL)
            sqf = work.tile([128, TOK], f32, tag="sq", name="t_sqf")
            rstdf = work.tile([1, TOK], f32, tag="rstd", name="t_rstdf")
            pssumf = ps.tile([1, TOK], f32, tag="pss", bufs=2, name="t_pssf")
            prstdf = ps.tile([128, TOK], f32, tag="prstd", name="t_prstdf")
            hf = work.tile([128, TOK], bf16, tag="hf", name="t_hf")
            for hh in range(2):
                r0 = HR[hh]
                nc.scalar.activation(out=sqf[:, r0:r0 + HTOK],
                                     in_=hps[:, r0:r0 + HTOK], func=AF.Square)
            for hh in range(2):
                r0 = HR[hh]
                nc.tensor.matmul(
                    out=pssumf[0:1, r0:r0 + HTOK], lhsT=onesD,
                    rhs=sqf[:, r0:r0 + HTOK].bitcast(f32r),
                    start=True, stop=True, skip_group_check=True)
            for hh in range(2):
                r0 = HR[hh]
                nc.vector.tensor_scalar(
                    out=rstdf[0:1, r0:r0 + HTOK], in0=pssumf[0:1, r0:r0 + HTOK],
                    scalar1=epsf, scalar2=-0.5, op0=OP.add, op1=OP.pow)
            for hh in range(2):
                r0 = HR[hh]
                nc.tensor.matmul(
                    out=prstdf[:, r0:r0 + HTOK], lhsT=sqrtDrow,
                    rhs=rstdf[0:1, r0:r0 + HTOK].bitcast(f32r),
                    start=True, stop=True, skip_group_check=True)
            for hh in range(2):
                r0 = HR[hh]
                nc.vector.scalar_tensor_tensor(
                    out=hf[:, r0:r0 + HTOK], in0=hps[:, r0:r0 + HTOK],
                    scalar=normf, in1=prstdf[:, r0:r0 + HTOK],
                    op0=OP.mult, op1=OP.mult)

            # ---------------- all-gather the features (bf16) ----------------
            ccin = dram.tile([128, TOK], bf16)
            nc.scalar.dma_start(out=ccin[:], in_=hf[:])
            # inner dim padded so the gathered blocks stay stride-separated
            TOKP = TOK + 8
            ccout = dram.tile([NCORES, 128, TOKP], bf16, addr_space="Shared")
            nc.gpsimd.collective_compute(
                "AllGather", OP.bypass,
                replica_groups=[list(range(NCORES))],
                ins=[ccin[:].opt()], outs=[ccout[:, :, 0:TOK]])
            # flatT[d, b, k] (k padded to 128; pads are zero), b = r*BLOC + b4
            flatT = const.tile([128, B, 128], bf16)
            nc.scalar.dma_start(
                out=flatT[:].rearrange("p (r x) k -> p r (x k)", r=NCORES),
                in_=bass.AP(tensor=ccout[:].tensor, offset=ccout[:].offset,
                            ap=[[TOKP, 128], [128 * TOKP, NCORES], [1, TOK]]),
            )
            fap = flatT[:]
            fp0 = list(fap.ap[0])

            # ---------------- streamed output matmul ----------------
            # y[b, o] accumulated over the 125 (k, d) chunks. Stationary
            # operand = flatT columns (j, b) at offset k: output row j*32+b
            # holds sum_d flat[d, b, k+j] * wt_k[d, o]; rows 0..31 / j=0 are
            # the real batch rows, the rest M-padding. Moving operand = the
            # streamed bf16 W tile.
            pyb0 = ps.tile([128, 512], f32, tag="pac", name="t_pyb0")
            pyb1 = ps.tile([128, OSL - 512], f32, tag="pbc", name="t_pyb1")
            pybs = ((pyb0, 0, 512), (pyb1, 512, OSL - 512))
            for q in range(NQ):
                kc_n = min(WKC, NP - q * WKC)
                wtl = wring.tile([128, WKC, OSL], bf16, tag="wt", name="t_wt")
                nc.sync.dma_start(
                    out=wtl[:, 0:kc_n, :],
                    in_=bass.AP(tensor=wtT, offset=q * WKC * 128 * OSL,
                                ap=[[OSL, 128], [128 * OSL, kc_n], [1, OSL]]),
                )
                for kc in range(kc_n):
                    k = q * WKC + kc
                    lhsT = bass.AP(tensor=fap.tensor, offset=fap.offset + k,
                                   ap=[fp0, [1, 4], [128, 32]])
                    for (pt, o0, on) in pybs:
                        nc.tensor.matmul(
                            out=pt[:], lhsT=lhsT,
                            rhs=wtl[:, kc, o0:o0 + on],
                            start=(k == 0), stop=(k == NP - 1),
                            skip_group_check=True)

            yout = work.tile([32, OSL], f32, tag="yout", name="t_yout")
            nc.scalar.copy(out=yout[:, 0:512], in_=pyb0[0:32, :])
            nc.scalar.copy(out=yout[:, 512:OSL], in_=pyb1[0:32, :])
            nc.scalar.dma_start(out=y[:], in_=yout[:])

    _legalize_waits(nc)
    return nc


def _legalize_waits(nc):
    """walrus on this toolchain accepts only one sync wait per non-sequencer
    instruction. Move extra waits onto standalone InstEventSemaphore
    instructions (sequencer-level waits, multi-wait legal) placed just
    before the owning instruction on the same engine."""
    n_moved = 0
    for bb in nc.main_func.blocks:
        out = []
        for inst in bb.instructions:
            si = inst.sync_info
            tn = type(inst).__name__
            if (si is not None and len(si.on_wait) > 1
                    and tn not in ("InstEventSemaphore", "InstNoOp")):
                waits = list(si.on_wait)
                for w in waits[:-1]:
                    ev = mybir.InstNoOp(
                        name=f"lw_{inst.name}_{n_moved}", ins=[], outs=[],
                        engine=inst.engine)
                    ev.sync_info = mybir.SyncInfo(on_wait=[w], on_update=[])
                    nc.register_instruction(ev)
                    out.append(ev)
                    n_moved += 1
                inst.sync_info = mybir.SyncInfo(
                    on_wait=[waits[-1]], on_update=list(si.on_update))
            out.append(inst)
        bb.instructions = out


def _sincos_pe(n, d):
    pos = np.arange(n, dtype=np.float32)[:, None]
    sin_cols, cos_cols = (d + 1) // 2, d // 2
    denom = d / 2.0
    sin_div = np.exp(
        (-math.log(10000.0) * np.arange(sin_cols, dtype=np.float32) / denom)
    ).astype(np.float32)
    cos_div = np.exp(
        (-math.log(10000.0) * np.arange(cos_cols, dtype=np.float32) / denom)
    ).astype(np.float32)
    pe = np.zeros((n, d), dtype=np.float32)
    pe[:, 0::2] = np.sin(pos * sin_div[None, :])
    pe[:, 1::2] = np.cos(pos * cos_div[None, :])
    return pe


def _to_bf16(a):
    import ml_dtypes
    return np.asarray(a, np.float32).astype(ml_dtypes.bfloat16)


def make_in_maps(x, pe_W, pe_b, norm_w, ipa_W, ipb_W, conv_W, conv_b,
                 alpha, beta, gamma, delta, op_W, normf_w, out_W, out_b):
    f = np.float32
    x = np.asarray(x, f)
    x_pad = np.zeros((B, C, LPAD + 4), f)
    x_pad[:, :, :L] = x
    # device layout: [p2*64+c, b_loc, l] with p2=1 rows shifted by 8 along l;
    # 129 windows of 4 (the last is zero padding for the shifted matmuls)
    xcT = x_pad.transpose(1, 0, 2)                     # [c, b, lpad]
    x_dev = np.empty((2, C, B, NW * 4), f)
    x_dev[0] = xcT[:, :, 0:NW * 4]
    x_dev[1] = xcT[:, :, 8:8 + NW * 4]
    x_dev = x_dev.reshape(128, B, NW * 4)

    pw = np.asarray(pe_W, f).reshape(D, C, P)          # [d, c, p]
    t = pw.transpose(1, 2, 0)                          # [c, p, d]
    pe8 = np.ascontiguousarray(
        t.reshape(C, 2, 8, D).transpose(2, 1, 0, 3).reshape(8, 128, 128))
    pe8 = np.ascontiguousarray(pe8.transpose(1, 0, 2))  # [pp, j, d]

    posb = _sincos_pe(NP, D).T + np.asarray(pe_b, f)[:, None]   # [128, 125]
    posbB = np.zeros((128, BLOC, 128), f)
    posbB[:, :, :NP] = posb[:, None, :]
    posbB = posbB.reshape(128, TOK)

    ident = np.eye(128, dtype=f)
    sqd = np.full((128, 128), SQRTD, f)
    oner = np.ones((128, HTOK), f)
    misc = np.zeros((128, 2), f)
    misc[:, 0] = 1.0
    misc[:, 1] = np.asarray(normf_w, f)

    nw = np.asarray(norm_w, f)                          # [NL, D]
    ipa = np.asarray(ipa_W, f)                          # [NL, INNER, D]
    cw = np.asarray(conv_W, f)[:, :, 0, :]              # [NL, INNER, K5]
    w5 = (ipa.transpose(0, 2, 1)[:, None, :, :]         # [NL, 1, D, INNER]
          * cw.transpose(0, 2, 1)[:, :, None, :]        # [NL, K5, 1, INNER]
          * nw[:, None, :, None])                       # [NL, K5, D, INNER]
    bwh = np.asarray(ipb_W, f).transpose(0, 2, 1) * nw[:, :, None]  # [NL, D, INNER]
    owh = np.asarray(op_W, f).transpose(0, 2, 1).reshape(NL, 2, 128, D)

    asig = 1.0 / (1.0 + np.exp(-np.asarray(alpha, np.float64)))
    gb = np.asarray(gamma, np.float64) * np.asarray(beta, np.float64)
    p4 = np.stack([np.asarray(conv_b, f),
                   asig.astype(f),
                   gb.astype(f),
                   np.asarray(delta, f)], axis=0)       # [4, NL, INNER]

    layer_cols = []
    for l in range(NL):
        w5l = w5[l].transpose(1, 0, 2).reshape(128, K5 * INNER)   # [d,(k i)]
        bwl = bwh[l].reshape(128, INNER)                          # [d, i]
        # residual lives in PSUM as h' = h * 2^-l: fold 2^-(l+1) into op_W
        owl = np.ascontiguousarray(
            owh[l].transpose(2, 0, 1)).reshape(128, 2 * D) * (0.5 ** (l + 1))
        scl = np.ascontiguousarray(
            p4[:, l].reshape(4, 2, 128).transpose(2, 0, 1)).reshape(128, 8)
        cbr = np.zeros((128, INNER), f)
        cbr[0] = np.asarray(conv_b, f)[l]
        layer_cols.append(np.concatenate(
            [w5l, bwl, owl.astype(f), scl, cbr], axis=1))

    oW = np.asarray(out_W, f)
    pe8h = _to_bf16(pe8.reshape(128, 8 * 128))
    wpk = np.ascontiguousarray(np.concatenate(
        [posbB, ident, sqd, oner, misc] + layer_cols, axis=1))
    assert wpk.shape[1] == WCOLS, wpk.shape
    in_maps = []
    for r in range(NCORES):
        wphk = np.concatenate([
            _to_bf16(x_dev[:, r * BLOC:(r + 1) * BLOC, :].reshape(
                128, BLOC * NW * 4)),
            pe8h,
        ], axis=1)
        assert wphk.shape[1] == HCOLS, wphk.shape
        in_maps.append({
            "wph": np.ascontiguousarray(wphk),
            "wpack": wpk,
            "wt": _to_bf16(oW[r * OSL:(r + 1) * OSL].T),
        })
    return in_maps


def kernel(**inputs):
    global _PROG
    if _PROG is None:
        _PROG = build_program()
    in_maps = make_in_maps(**inputs)
    res = run_bass_kernel_spmd(_PROG, in_maps, list(range(NCORES)))
    return assemble_output([res.results[r]["y"] for r in range(NCORES)],
                           inputs["out_b"])


def assemble_output(ys, out_b=None):
    y = np.concatenate([np.asarray(yr).reshape(B, OSL) for yr in ys], axis=1)
    if out_b is not None:
        y = y + np.asarray(out_b, np.float32)[None, :]
    return y.reshape(B, C, F)


# revision 34
# speedup vs baseline: 1.0693x; 1.0440x over previous
"""Trainium2 Bass kernel for nn_CMambaSlim.

Strategy (8 NeuronCores):
  - Data-parallel trunk: each core runs the CMamba trunk (patch embed, 4
    mamba blocks, final RMSNorm) on B/8 = 4 batch samples, fp32/f32r.
  - AllGather of the flattened features (1 MB bf16) across the 8 cores.
  - Tensor-parallel output layer: core r streams rows [r*768, (r+1)*768) of
    out_W in bf16 (pre-transposed on host to [16000, 768]) and computes
    y[:, r*768:(r+1)*768]. out_b is added on the host during unsharding.

Schedule notes (CoreSim cost model):
  - All DMAs serialize on the DMA-engine device and hold the issuing
    engine's sequencer. SP's queue carries ONLY the wpack loads + the 32
    streamed weight chunks, so weight prefetch runs through the trunk and
    the AllGather. ccin/flatT/y DMAs issue from the Activation engine.
  - The residual stream h lives in PSUM: the out-projection matmuls
    accumulate straight into it (h' = h * 2^-l with the 2^-(l+1) folded
    into op_W host-side; rmsnorm is scale-invariant so only the eps
    constant needs a per-layer 4^-l).
  - The trunk is split into two independent 256-token halves (the conv
    windows are separated by a zeroed 4-column gap in hn), pipelined
    through ACT (square/sigmoid/aM), DVE (rstd/hn/u/gg/dab), Pool
    (scan/g0, SBUF-only operands), and PE.
  - ACT stays on the sigmoid table the whole trunk; rstd is computed on
    DVE as (ssum + D*eps_l)^-0.5 via AluOp.pow.
"""

import math
import os
import sys

import numpy as np

for _p in ("/opt/trn_rl_repo", "/root/.axon_site/_ro/trn_rl_repo"):
    if os.path.isdir(_p) and _p not in sys.path:
        sys.path.insert(0, _p)
        break

import concourse.bass as bass
import concourse.tile as tile
from concourse import mybir
from concourse.bass_utils import run_bass_kernel_spmd

# Model dims (hardcoded per problem spec)
B, C, L = 32, 64, 512
P, S = 16, 4
NP = 125
D = 128
INNER = 256
K5 = 5
NL = 4
F = 96
EPS = 1e-5

NCORES = 8
BLOC = B // NCORES            # 4 samples per core
OSL = (C * F) // NCORES       # 768 output cols per core
TOK = BLOC * 128              # padded token span (125 valid + 3 pad per sample)
HTOK = TOK // 2               # 256 tokens per pipelined half
HNW = 2 + HTOK + 4 + HTOK + 2  # hn with guards + inter-half gap = 520
LPAD = 520                    # x padded along L so the +8-shifted copy stays in bounds
NF = NP * D                   # 16000 contraction size
WKC = 4                       # k-chunks per weight-stream DMA
NQ = (NP + WKC - 1) // WKC    # 32 chunks (31 full + 1 partial)
WRING = 18                    # weight ring depth (chunks prefetchable)

f32 = mybir.dt.float32
f32r = mybir.dt.float32r
bf16 = mybir.dt.bfloat16
AF = mybir.ActivationFunctionType
OP = mybir.AluOpType

_PROG = None

SQRTD = math.sqrt(float(D))

# wph: bf16 embed inputs (x windows + patch-embed W), loaded first
NW = 129                                   # x windows (last is zero padding)
HOFF_PE8 = BLOC * NW * 4                   # 2064 cols of x
HCOLS = HOFF_PE8 + 8 * 128                 # + 1024 cols of patch-embed W
# wpack: fp32 consts (embed tail + one DMA per layer)
WOFF_IDN = TOK                             # 512 cols: posb broadcast to (b,k)
WOFF_SQD = WOFF_IDN + 128                  # 128 cols: identity matrix
WOFF_ONER = WOFF_SQD + 128                 # 128 cols: sqrt(D) everywhere
WOFF_MISC = WOFF_ONER + HTOK               # 256 cols: 1.0 (ones row)
WOFF_L0 = WOFF_MISC + 2                    # col 0: 1.0 (onesD), col 1: normf_w
LCOLS = K5 * INNER + INNER + 2 * D + 8 + INNER  # + 256 cols conv_b row
WCOLS = WOFF_L0 + NL * LCOLS


def build_program():
    nc = bass.Bass(num_devices=NCORES)

    wph = nc.declare_dram_parameter("wph", [128, HCOLS], bf16, isOutput=False)
    wpack = nc.declare_dram_parameter("wpack", [128, WCOLS], f32, isOutput=False)
    wt = nc.declare_dram_parameter("wt", [NF, OSL], bf16, isOutput=False)
    y = nc.declare_dram_parameter("y", [B, OSL], f32, isOutput=True)

    wtT = wt[:].tensor

    with tile.TileContext(nc) as tc:
        with (
            tc.tile_pool(name="const", bufs=1) as const,
            tc.tile_pool(name="work", bufs=1) as work,
            tc.tile_pool(name="wring", bufs=WRING) as wring,
            tc.tile_pool(name="ps", bufs=1, space="PSUM") as ps,
            tc.tile_pool(name="dram", bufs=1, space="DRAM") as dram,
        ):
            # ---------------- constant loads (embed parts, then per layer) ---
            # posbB/ident first (gates the first embed matmul), then x/pe8
            wp = const.tile([128, WCOLS], f32r)
            nc.sync.dma_start(out=wp[:, 0:WOFF_L0],
                              in_=wpack[:, 0:WOFF_L0].bitcast(f32r))
            wh = const.tile([128, HCOLS], bf16)
            nc.sync.dma_start(out=wh[:], in_=wph[:])
            for l in range(NL):
                c0 = WOFF_L0 + l * LCOLS
                nc.sync.dma_start(out=wp[:, c0:c0 + LCOLS],
                                  in_=wpack[:, c0:c0 + LCOLS].bitcast(f32r))

            xO4 = wh[:, 0:HOFF_PE8].rearrange(
                "p (b k s) -> p b k s", b=BLOC, s=4)          # [128, 4, 129, 4]
            pe8sb = wh[:, HOFF_PE8:HCOLS].rearrange("p (j d) -> p j d", j=8)
            posbB = wp[:, 0:WOFF_IDN]                          # [128, 512] (b,k)
            ident = wp[:, WOFF_IDN:WOFF_SQD]                   # I_128
            sqrtDrow = wp[0:1, WOFF_SQD:WOFF_SQD + 128]        # value sqrt(D)
            onesrow = wp[0:1, WOFF_ONER:WOFF_ONER + HTOK]      # value 1.0
            onesD = wp[:, WOFF_MISC:WOFF_MISC + 1]             # value 1.0
            normf = wp[:, WOFF_MISC + 1:WOFF_MISC + 2].bitcast(f32)

            def lview(l):
                b0 = WOFF_L0 + l * LCOLS
                w5 = wp[:, b0:b0 + K5 * INNER].rearrange(
                    "p (k i) -> p k i", k=K5)
                bw = wp[:, b0 + K5 * INNER:b0 + K5 * INNER + INNER]
                ow = wp[:, b0 + K5 * INNER + INNER:
                        b0 + K5 * INNER + INNER + 2 * D].rearrange(
                    "p (c d) -> p c d", c=2)
                scal = wp[:, b0 + LCOLS - 8 - INNER:
                          b0 + LCOLS - INNER].bitcast(f32).rearrange(
                    "p (s c) -> p s c", s=4)
                cbrow = wp[0:1, b0 + LCOLS - INNER:b0 + LCOLS]  # conv_b row
                return w5, bw, ow, scal, cbrow

            # mask01: 1 everywhere, 0 at each sample's k=0 column (scan reset)
            mask01 = const.tile([128, TOK], f32)
            nc.vector.memset(mask01[:], 1.0)
            for bq in range(BLOC):
                nc.vector.memset(mask01[:, bq * 128:bq * 128 + 1], 0.0)

            # residual stream h' lives in PSUM; out-projections accumulate
            # into it (never stopped). h' = h * 2^-l, exact via scaled op_W.
            hps = ps.tile([128, TOK], f32, tag="h", name="t_h")
            hps_bk = hps[:].rearrange("p (b k) -> p b k", b=BLOC)

            # normalized-input tile: [2 guard | half0 | 4 gap | half1 | 2 guard]
            hn = const.tile([128, HNW], f32)
            nc.vector.memset(hn[:, 0:2], 0.0)
            nc.vector.memset(hn[:, 2 + HTOK:2 + HTOK + 4], 0.0)
            nc.vector.memset(hn[:, HNW - 2:], 0.0)
            hnr = hn[:].bitcast(f32r)
            HNS = (2, 2 + HTOK + 4)        # hn write offset per half
            HR = (0, HTOK)                 # token-range start per half

            # ---------------- patch embedding (into h PSUM) ----------------
            # identity @ posbB first: start=True pending-zeros the whole
            # region and this matmul touches every byte.
            nc.tensor.matmul(out=hps[:], lhsT=ident, rhs=posbB,
                             start=True, stop=False, skip_group_check=True)
            for j in range(8):
                jq, jr = j // 4, j % 4
                rhs = xO4[:, :, jq:jq + 128, jr]
                nc.tensor.matmul(
                    out=hps[:], lhsT=pe8sb[:, j, :],
                    rhs=rhs, start=False, stop=False, skip_group_check=True)
            # zero the 3 pad tokens per sample (windows 125..127 hold junk)
            nc.vector.memset(hps_bk[:, :, 125:128], 0.0)

            # ---------------- mamba layers (two pipelined halves) -----------
            def emit_layer(l):
                w5sb, bwsb, owsb, scalsb, cbrow = lview(l)
                epsl = float(D) * EPS * (0.25 ** l)
                sq = work.tile([128, TOK], f32, tag="sq", name=f"sq_{l}")
                rstd = work.tile([1, TOK], f32, tag="rstd", name=f"rstd_{l}")
                pssum = ps.tile([1, TOK], f32, tag="pss", bufs=2, name=f"pss_{l}")
                prstd = ps.tile([128, TOK], f32, tag="prstd", name=f"prstd_{l}")
                pa = [ps.tile([128, TOK], f32, tag=f"pa{ic}", name=f"pa{ic}_{l}")
                      for ic in range(2)]
                pb = [ps.tile([128, TOK], f32, tag=f"pb{ic}", name=f"pb{ic}_{l}")
                      for ic in range(2)]
                aM, sg, ab, sc, dab, gg = ({}, {}, {}, {}, {}, {})
                for ic in range(2):
                    aM[ic] = work.tile([128, TOK], f32, tag=f"am{ic}", bufs=2,
                                       name=f"am{ic}_{l}")
                    sg[ic] = work.tile([128, TOK], f32, tag=f"sg{ic}", name=f"sg{ic}_{l}")
                    ab[ic] = work.tile([128, TOK], f32, tag=f"ab{ic}", name=f"ab{ic}_{l}")
                    sc[ic] = work.tile([128, TOK], f32, tag=f"s{ic}", name=f"s{ic}_{l}")
                    dab[ic] = work.tile([128, TOK], f32, tag=f"dab{ic}", name=f"dab{ic}_{l}")
                    gg[ic] = work.tile([128, TOK], f32, tag=f"g{ic}", name=f"g{ic}_{l}")

                # aM first: depends only on constants, fills ACT while the
                # previous layer's gate phase runs (bufs=2 on the am tags)
                for hh in range(2):
                    r0 = HR[hh]
                    for ic in range(2):
                        nc.scalar.activation(
                            out=aM[ic][:, r0:r0 + HTOK], in_=mask01[:, r0:r0 + HTOK],
                            func=AF.Copy, scale=scalsb[:, 1, ic:ic + 1])
                for hh in range(2):
                    r0 = HR[hh]
                    nc.scalar.activation(out=sq[:, r0:r0 + HTOK],
                                         in_=hps[:, r0:r0 + HTOK], func=AF.Square)
                for hh in range(2):
                    r0 = HR[hh]
                    nc.tensor.matmul(
                        out=pssum[0:1, r0:r0 + HTOK], lhsT=onesD,
                        rhs=sq[:, r0:r0 + HTOK].bitcast(f32r),
                        start=True, stop=True, skip_group_check=True)
                for hh in range(2):
                    r0 = HR[hh]
                    nc.vector.tensor_scalar(
                        out=rstd[0:1, r0:r0 + HTOK], in0=pssum[0:1, r0:r0 + HTOK],
                        scalar1=epsl, scalar2=-0.5, op0=OP.add, op1=OP.pow)
                for hh in range(2):
                    r0 = HR[hh]
                    nc.tensor.matmul(
                        out=prstd[:, r0:r0 + HTOK], lhsT=sqrtDrow,
                        rhs=rstd[0:1, r0:r0 + HTOK].bitcast(f32r),
                        start=True, stop=True, skip_group_check=True)
                for hh in range(2):
                    r0, h0 = HR[hh], HNS[hh]
                    nc.vector.tensor_tensor(
                        out=hn[:, h0:h0 + HTOK], in0=hps[:, r0:r0 + HTOK],
                        in1=prstd[:, r0:r0 + HTOK], op=OP.mult)
                for hh in range(2):
                    h0 = HNS[hh]
                    for ic in range(2):
                        for dk in range(K5):
                            nc.tensor.matmul(
                                out=pa[ic][:, HR[hh]:HR[hh] + HTOK],
                                lhsT=w5sb[:, dk, ic * 128:(ic + 1) * 128],
                                rhs=hnr[:, h0 - 2 + dk:h0 - 2 + dk + HTOK],
                                start=(dk == 0), stop=(dk == K5 - 1),
                                skip_group_check=True)
                    for ic in range(2):
                        nc.tensor.matmul(
                            out=pb[ic][:, HR[hh]:HR[hh] + HTOK],
                            lhsT=bwsb[:, ic * 128:(ic + 1) * 128],
                            rhs=hnr[:, h0:h0 + HTOK],
                            start=True, stop=True, skip_group_check=True)
                # gate phase: half-major so the two halves pipeline cleanly
                for hh in range(2):
                    r0 = HR[hh]
                    for ic in range(2):
                        # silu(z) = z * sigmoid(z), z = conv + conv_b
                        nc.scalar.activation(
                            out=sg[ic][:, r0:r0 + HTOK], in_=pa[ic][:, r0:r0 + HTOK],
                            func=AF.Sigmoid, bias=scalsb[:, 0, ic:ic + 1], scale=1.0)
                    for ic in range(2):
                        nc.vector.scalar_tensor_tensor(
                            out=ab[ic][:, r0:r0 + HTOK], in0=pa[ic][:, r0:r0 + HTOK],
                            scalar=scalsb[:, 0, ic:ic + 1],
                            in1=sg[ic][:, r0:r0 + HTOK], op0=OP.add, op1=OP.mult)
                    for ic in range(2):
                        # scan: state = aM*state + u (Pool; SBUF operands only)
                        nc.gpsimd.tensor_tensor_scan(
                            out=sc[ic][:, r0:r0 + HTOK], data0=aM[ic][:, r0:r0 + HTOK],
                            data1=ab[ic][:, r0:r0 + HTOK], initial=0.0,
                            op0=OP.mult, op1=OP.add)
                    for ic in range(2):
                        nc.vector.tensor_scalar_mul(
                            out=dab[ic][:, r0:r0 + HTOK], in0=ab[ic][:, r0:r0 + HTOK],
                            scalar1=scalsb[:, 3, ic:ic + 1])
                    for ic in range(2):
                        # g0 = gamma*beta*s + dab (Pool; SBUF operands only)
                        nc.gpsimd.scalar_tensor_tensor(
                            out=sc[ic][:, r0:r0 + HTOK], in0=sc[ic][:, r0:r0 + HTOK],
                            scalar=scalsb[:, 2, ic:ic + 1],
                            in1=dab[ic][:, r0:r0 + HTOK], op0=OP.mult, op1=OP.add)
                    for ic in range(2):
                        # pads stay zero: hn pads are zero so pb pads are zero
                        nc.vector.tensor_tensor(
                            out=gg[ic][:, r0:r0 + HTOK], in0=sc[ic][:, r0:r0 + HTOK],
                            in1=pb[ic][:, r0:r0 + HTOK], op=OP.mult)
                    for ic in range(2):
                        # residual: h' += 2^-(l+1) * oW @ g (scale folded into oW)
                        nc.tensor.matmul(
                            out=hps[:, r0:r0 + HTOK], lhsT=owsb[:, ic, :],
                            rhs=gg[ic][:, r0:r0 + HTOK].bitcast(f32r),
                            start=False, stop=False, skip_group_check=True)

            for l in range(NL):
                emit_layer(l)

            # ---------------- final rmsnorm ----------------
            epsf = float(D) * EPS * (0.25 ** NL)
            sqf = work.tile([128, TOK], f32, tag="sq", name="t_sqf")
            rstdf = work.tile([1, TOK], f32, tag="rstd", name="t_rstdf")
            pssumf = ps.tile([1, TOK], f32, tag="pss", bufs=2, name="t_pssf")
            prstdf = ps.tile([128, TOK], f32, tag="prstd", name="t_prstdf")
            hf = work.tile([128, TOK], bf16, tag="hf", name="t_hf")
            for hh in range(2):
                r0 = HR[hh]
                nc.scalar.activation(out=sqf[:, r0:r0 + HTOK],
                                     in_=hps[:, r0:r0 + HTOK], func=AF.Square)
            for hh in range(2):
                r0 = HR[hh]
                nc.tensor.matmul(
                    out=pssumf[0:1, r0:r0 + HTOK], lhsT=onesD,
                    rhs=sqf[:, r0:r0 + HTOK].bitcast(f32r),
                    start=True, stop=True, skip_group_check=True)
            for hh in range(2):
                r0 = HR[hh]
                nc.vector.tensor_scalar(
                    out=rstdf[0:1, r0:r0 + HTOK], in0=pssumf[0:1, r0:r0 + HTOK],
                    scalar1=epsf, scalar2=-0.5, op0=OP.add, op1=OP.pow)
            for hh in range(2):
                r0 = HR[hh]
                nc.tensor.matmul(
                    out=prstdf[:, r0:r0 + HTOK], lhsT=sqrtDrow,
                    rhs=rstdf[0:1, r0:r0 + HTOK].bitcast(f32r),
                    start=True, stop=True, skip_group_check=True)
            for hh in range(2):
                r0 = HR[hh]
                nc.vector.scalar_tensor_tensor(
                    out=hf[:, r0:r0 + HTOK], in0=hps[:, r0:r0 + HTOK],
                    scalar=normf, in1=prstdf[:, r0:r0 + HTOK],
                    op0=OP.mult, op1=OP.mult)

            # ---------------- all-gather the features (bf16) ----------------
            ccin = dram.tile([128, TOK], bf16)
            nc.scalar.dma_start(out=ccin[:], in_=hf[:])
            # inner dim padded so the gathered blocks stay stride-separated
            TOKP = TOK + 8
            ccout = dram.tile([NCORES, 128, TOKP], bf16, addr_space="Shared")
            nc.gpsimd.collective_compute(
                "AllGather", OP.bypass,
                replica_groups=[list(range(NCORES))],
                ins=[ccin[:].opt()], outs=[ccout[:, :, 0:TOK]])
            # flatT[d, b, k] (k padded to 128; pads are zero), b = r*BLOC + b4
            flatT = const.tile([128, B, 128], bf16)
            nc.scalar.dma_start(
                out=flatT[:].rearrange("p (r x) k -> p r (x k)", r=NCORES),
                in_=bass.AP(tensor=ccout[:].tensor, offset=ccout[:].offset,
                            ap=[[TOKP, 128], [128 * TOKP, NCORES], [1, TOK]]),
            )
            fap = flatT[:]
            fp0 = list(fap.ap[0])

            # ---------------- streamed output matmul ----------------
            # y[b, o] accumulated over the 125 (k, d) chunks. Stationary
            # operand = flatT columns (j, b) at offset k: output row j*32+b
            # holds sum_d flat[d, b, k+j] * wt_k[d, o]; rows 0..31 / j=0 are
            # the real batch rows, the rest M-padding. Moving operand = the
            # streamed bf16 W tile.
            pyb0 = ps.tile([128, 512], f32, tag="pa0", name="t_pyb0")
            pyb1 = ps.tile([128, OSL - 512], f32, tag="pa1", name="t_pyb1")
            pybs = ((pyb0, 0, 512), (pyb1, 512, OSL - 512))
            for q in range(NQ):
                kc_n = min(WKC, NP - q * WKC)
                wtl = wring.tile([128, WKC, OSL], bf16, tag="wt", name="t_wt")
                nc.sync.dma_start(
                    out=wtl[:, 0:kc_n, :],
                    in_=bass.AP(tensor=wtT, offset=q * WKC * 128 * OSL,
                                ap=[[OSL, 128], [128 * OSL, kc_n], [1, OSL]]),
                )
                for kc in range(kc_n):
                    k = q * WKC + kc
                    lhsT = bass.AP(tensor=fap.tensor, offset=fap.offset + k,
                                   ap=[fp0, [1, 4], [128, 32]])
                    for (pt, o0, on) in pybs:
                        nc.tensor.matmul(
                            out=pt[:], lhsT=lhsT,
                            rhs=wtl[:, kc, o0:o0 + on],
                            start=(k == 0), stop=(k == NP - 1),
                            skip_group_check=True)

            yout = work.tile([32, OSL], f32, tag="yout", name="t_yout")
            nc.scalar.copy(out=yout[:, 0:512], in_=pyb0[0:32, :])
            nc.scalar.copy(out=yout[:, 512:OSL], in_=pyb1[0:32, :])
            nc.scalar.dma_start(out=y[:], in_=yout[:])

    _legalize_waits(nc)
    return nc


def _legalize_waits(nc):
    """walrus on this toolchain accepts only one sync wait per non-sequencer
    instruction. Move extra waits onto standalone InstEventSemaphore
    instructions (sequencer-level waits, multi-wait legal) placed just
    before the owning instruction on the same engine."""
    n_moved = 0
    for bb in nc.main_func.blocks:
        out = []
        for inst in bb.instructions:
            si = inst.sync_info
            tn = type(inst).__name__
            if (si is not None and len(si.on_wait) > 1
                    and tn not in ("InstEventSemaphore", "InstNoOp")):
                waits = list(si.on_wait)
                for w in waits[:-1]:
                    ev = mybir.InstNoOp(
                        name=f"lw_{inst.name}_{n_moved}", ins=[], outs=[],
                        engine=inst.engine)
                    ev.sync_info = mybir.SyncInfo(on_wait=[w], on_update=[])
                    nc.register_instruction(ev)
                    out.append(ev)
                    n_moved += 1
                inst.sync_info = mybir.SyncInfo(
                    on_wait=[waits[-1]], on_update=list(si.on_update))
            out.append(inst)
        bb.instructions = out


def _sincos_pe(n, d):
    pos = np.arange(n, dtype=np.float32)[:, None]
    sin_cols, cos_cols = (d + 1) // 2, d // 2
    denom = d / 2.0
    sin_div = np.exp(
        (-math.log(10000.0) * np.arange(sin_cols, dtype=np.float32) / denom)
    ).astype(np.float32)
    cos_div = np.exp(
        (-math.log(10000.0) * np.arange(cos_cols, dtype=np.float32) / denom)
    ).astype(np.float32)
    pe = np.zeros((n, d), dtype=np.float32)
    pe[:, 0::2] = np.sin(pos * sin_div[None, :])
    pe[:, 1::2] = np.cos(pos * cos_div[None, :])
    return pe


def _to_bf16(a):
    import ml_dtypes
    return np.asarray(a, np.float32).astype(ml_dtypes.bfloat16)


def make_in_maps(x, pe_W, pe_b, norm_w, ipa_W, ipb_W, conv_W, conv_b,
                 alpha, beta, gamma, delta, op_W, normf_w, out_W, out_b):
    f = np.float32
    x = np.asarray(x, f)
    x_pad = np.zeros((B, C, LPAD + 4), f)
    x_pad[:, :, :L] = x
    # device layout: [p2*64+c, b_loc, l] with p2=1 rows shifted by 8 along l;
    # 129 windows of 4 (the last is zero padding for the shifted matmuls)
    xcT = x_pad.transpose(1, 0, 2)                     # [c, b, lpad]
    x_dev = np.empty((2, C, B, NW * 4), f)
    x_dev[0] = xcT[:, :, 0:NW * 4]
    x_dev[1] = xcT[:, :, 8:8 + NW * 4]
    x_dev = x_dev.reshape(128, B, NW * 4)

    pw = np.asarray(pe_W, f).reshape(D, C, P)          # [d, c, p]
    t = pw.transpose(1, 2, 0)                          # [c, p, d]
    pe8 = np.ascontiguousarray(
        t.reshape(C, 2, 8, D).transpose(2, 1, 0, 3).reshape(8, 128, 128))
    pe8 = np.ascontiguousarray(pe8.transpose(1, 0, 2))  # [pp, j, d]

    posb = _sincos_pe(NP, D).T + np.asarray(pe_b, f)[:, None]   # [128, 125]
    posbB = np.zeros((128, BLOC, 128), f)
    posbB[:, :, :NP] = posb[:, None, :]
    posbB = posbB.reshape(128, TOK)

    ident = np.eye(128, dtype=f)
    sqd = np.full((128, 128), SQRTD, f)
    oner = np.ones((128, HTOK), f)
    misc = np.zeros((128, 2), f)
    misc[:, 0] = 1.0
    misc[:, 1] = np.asarray(normf_w, f)

    nw = np.asarray(norm_w, f)                          # [NL, D]
    ipa = np.asarray(ipa_W, f)                          # [NL, INNER, D]
    cw = np.asarray(conv_W, f)[:, :, 0, :]              # [NL, INNER, K5]
    w5 = (ipa.transpose(0, 2, 1)[:, None, :, :]         # [NL, 1, D, INNER]
          * cw.transpose(0, 2, 1)[:, :, None, :]        # [NL, K5, 1, INNER]
          * nw[:, None, :, None])                       # [NL, K5, D, INNER]
    bwh = np.asarray(ipb_W, f).transpose(0, 2, 1) * nw[:, :, None]  # [NL, D, INNER]
    owh = np.asarray(op_W, f).transpose(0, 2, 1).reshape(NL, 2, 128, D)

    asig = 1.0 / (1.0 + np.exp(-np.asarray(alpha, np.float64)))
    gb = np.asarray(gamma, np.float64) * np.asarray(beta, np.float64)
    p4 = np.stack([np.asarray(conv_b, f),
                   asig.astype(f),
                   gb.astype(f),
                   np.asarray(delta, f)], axis=0)       # [4, NL, INNER]

    layer_cols = []
    for l in range(NL):
        w5l = w5[l].transpose(1, 0, 2).reshape(128, K5 * INNER)   # [d,(k i)]
        bwl = bwh[l].reshape(128, INNER)                          # [d, i]
        # residual lives in PSUM as h' = h * 2^-l: fold 2^-(l+1) into op_W
        owl = np.ascontiguousarray(
            owh[l].transpose(2, 0, 1)).reshape(128, 2 * D) * (0.5 ** (l + 1))
        scl = np.ascontiguousarray(
            p4[:, l].reshape(4, 2, 128).transpose(2, 0, 1)).reshape(128, 8)
        cbr = np.zeros((128, INNER), f)
        cbr[0] = np.asarray(conv_b, f)[l]
        layer_cols.append(np.concatenate(
            [w5l, bwl, owl.astype(f), scl, cbr], axis=1))

    oW = np.asarray(out_W, f)
    pe8h = _to_bf16(pe8.reshape(128, 8 * 128))
    wpk = np.ascontiguousarray(np.concatenate(
        [posbB, ident, sqd, oner, misc] + layer_cols, axis=1))
    assert wpk.shape[1] == WCOLS, wpk.shape
    in_maps = []
    for r in range(NCORES):
        wphk = np.concatenate([
            _to_bf16(x_dev[:, r * BLOC:(r + 1) * BLOC, :].reshape(
                128, BLOC * NW * 4)),
            pe8h,
        ], axis=1)
        assert wphk.shape[1] == HCOLS, wphk.shape
        in_maps.append({
            "wph": np.ascontiguousarray(wphk),
            "wpack": wpk,
            "wt": _to_bf16(oW[r * OSL:(r + 1) * OSL].T),
        })
    return in_maps


def kernel(**inputs):
    global _PROG
    if _PROG is None:
        _PROG = build_program()
    in_maps = make_in_maps(**inputs)
    res = run_bass_kernel_spmd(_PROG, in_maps, list(range(NCORES)))
    return assemble_output([res.results[r]["y"] for r in range(NCORES)],
                           inputs["out_b"])


def assemble_output(ys, out_b=None):
    y = np.concatenate([np.asarray(yr).reshape(B, OSL) for yr in ys], axis=1)
    if out_b is not None:
        y = y + np.asarray(out_b, np.float32)[None, :]
    return y.reshape(B, C, F)
